# revision 7
# baseline (speedup 1.0000x reference)
"""Causal single-head attention (B=4, S=2048, D=1024) on 8 TRN2 NeuronCores.

fp8(e4m3) DoubleRow rewrite of the bf16 baseline.  Same sharding: core
c -> (batch b = c//2, half h = c%2); 8 query slots of 128 rows with padded
causal key-lengths L_s = 256*(s+1); scores computed transposed; Wqk = Wq@Wk^T
fused host-side; out = ((attn @ X_v) @ W_v) / den.

Quantization scheme (all matmuls fp8e4 DoubleRow, PSUM fp32):
  G = Xq @ Wqk      single-fp8 (Wqk hi only, Xq fp8); G -> qt hi+lo on
                    device (ACT copy + DVE scalar_tensor_tensor residual).
  scores = qt@K^T   2-product: qt hi+lo (device), K^T single fp8 (host).
  U = attn @ Xv     3-product: attn hi+lo (ACT exp f8 + bf16, DVE sub),
                    Xv hi+lo (host).  U -> ut hi+lo on device.
  Y = U @ Wv        3-product: ut hi+lo (device), Wv hi+lo (host).
hi+lo fp8 pairs represent bf16 values exactly, so the residual GEMMs have
~bf16-level error; the 2-product GEMMs err only by the single side's fp8
quantization.  The softmax denominator is summed from the quantized bf16
attn (== hi+lo exactly), cancelling common-mode quantization error; the
"ones" den vector carries value au*aw so 1/den absorbs all output scales.
Scales are powers of two shipped in a [P,4] f32 vector, so the compiled
program is input-independent.

Schedule: the cost model serializes all DMA on one shared resource
(~360 GB/s) with ~0.6us per-issue overhead, so all bulk input DMA goes on
ONE queue (sync) in exact first-consumption order; consts + Xq ride the
scalar queue; output DMA on the SWDGE.  Slots run ASCENDING (0..7) so K/V
chunks stream just-in-time, and the 8 Y GEMMs are deferred to the end
(ut hi/lo tiles are tiny and stay resident) where they keep the PE busy
through the tail while Wv has long arrived.

NOTE (hazard, empirical): interleaving start/stop matmul accumulation chains
across sub-regions of a single PSUM bank corrupts results on HW; keep each
sub-tile's chain contiguous (interleaving across banks is fine).
"""

import numpy as np

import concourse.bacc as bacc
import concourse.mybir as mybir
import concourse.tile as tile
from concourse import bass_utils

B, S, D = 4, 2048, 1024
P = 128
DC = D // P          # 8 contraction chunks
DCP = DC // 2        # 4 DoubleRow contraction pairs
EC = D // P
NSLOT = 8
NQ = NSLOT * P
SCALE = 1.0 / float(np.sqrt(np.float32(S)))
NEG = -1.0e9

F32 = mybir.dt.float32
BF16 = mybir.dt.bfloat16
F8 = mybir.dt.float8e4
DR = mybir.MatmulPerfMode.DoubleRow
MULT = mybir.AluOpType.mult
SUB = mybir.AluOpType.subtract


def build_attention_nc():
    nc = bacc.Bacc("TRN2", target_bir_lowering=False)

    xq_in = nc.dram_tensor("xq", [P, DC, NQ], F8, kind="ExternalInput")
    xk_in = nc.dram_tensor("xk", [P, EC, S], F8, kind="ExternalInput")
    # X_v hi/lo interleaved per key tile: [p, tile, {hi,lo}, d]
    xv_in = nc.dram_tensor("xv", [P, S // P, 2, D], F8, kind="ExternalInput")
    # Wqk packed in consumption-ordered chunks: chunk c = (colhalf a, dcpair
    # dp) at c = a*4+dp, holding [{row0,row1}, 512 cols] (hi only; the G
    # GEMM runs single-fp8 -- Xq and Wqk each quantized once).
    wq_in = nc.dram_tensor("wq", [P, 8, 2, 512], F8, kind="ExternalInput")
    wvh_in = nc.dram_tensor("wvh", [P, DC, D], F8, kind="ExternalInput")
    wvl_in = nc.dram_tensor("wvl", [P, DC, D], F8, kind="ExternalInput")
    mask_a_in = nc.dram_tensor("mask_a", [P, P], BF16, kind="ExternalInput")
    mask_b_in = nc.dram_tensor("mask_b", [P, P], BF16, kind="ExternalInput")
    ones_in = nc.dram_tensor("ones", [P, 1], BF16, kind="ExternalInput")
    scl_in = nc.dram_tensor("scl", [P, 4], F32, kind="ExternalInput")
    out = nc.dram_tensor("out", [NQ, D], BF16, kind="ExternalOutput")

    with tile.TileContext(nc) as tc:
        with (
            tc.tile_pool(name="res", bufs=1) as res,
            tc.tile_pool(name="psum", bufs=4, space="PSUM") as pp,
        ):
            kt_sb = res.tile([P, EC, S], F8)        # K^T  [e, keys]
            xv_sb = res.tile([P, S // P, 2, D], F8)  # X_v hi/lo interleaved
            xq_sb = res.tile([P, DC, NQ], F8)       # X_q^T [d, q]
            qth_sb = res.tile([P, EC, NQ], F8)      # Q^T hi [e, q]
            qtl_sb = res.tile([P, EC, NQ], F8)      # Q^T lo
            wq_sb = res.tile([P, 8, 2, 512], F8)  # Wqk packed chunks
            wvh_sb = res.tile([P, DC, D], F8)
            wvl_sb = res.tile([P, DC, D], F8)
            uth_sb = res.tile([P, NSLOT, DC, P], F8)  # U^T hi (all slots)
            utl_sb = res.tile([P, NSLOT, DC, P], F8)
            rec_sb = res.tile([P, NSLOT], F32)        # 1/(c*den) per slot
            mask_a = res.tile([P, P], BF16)
            mask_b = res.tile([P, P], BF16)
            ones_sb = res.tile([P, 1], BF16)
            scl_sb = res.tile([P, 4], F32)
            s_exp = scl_sb[:, 0:1]   # exp scale = SCALE/(ag*ak)
            s_qt = scl_sb[:, 1:2]    # gamma_g: psum->qt scale
            s_ut = scl_sb[:, 2:3]    # gamma_u: psum->ut scale

            # ---- DMA: Xq-qb0 + consts on scalar; all bulk on sync, in
            # first-consumption order.  Each issue costs ~0.63us on its
            # queue, so the stream uses few, large chunks. ----
            nc.scalar.dma_start(xq_sb[:, 0:2, 0:512], xq_in[:, 0:2, 0:512])
            nc.gpsimd.dma_start(xq_sb[:, 2:8, 0:512], xq_in[:, 2:8, 0:512])
            nc.gpsimd.dma_start(scl_sb, scl_in[:, :])
            nc.gpsimd.dma_start(ones_sb, ones_in[:, :])
            nc.gpsimd.dma_start(mask_a, mask_a_in[:, :])
            nc.gpsimd.dma_start(mask_b, mask_b_in[:, :])
            nc.sync.dma_start(wq_sb[:, 0:1], wq_in[:, 0:1])
            nc.sync.dma_start(wq_sb[:, 1:4], wq_in[:, 1:4])
            nc.sync.dma_start(wq_sb[:, 4:8], wq_in[:, 4:8])
            nc.sync.dma_start(kt_sb[:, :, 0:512], xk_in[:, :, 0:512])
            nc.sync.dma_start(xv_sb[:, 0:2], xv_in[:, 0:2])
            nc.sync.dma_start(kt_sb[:, :, 512:1024], xk_in[:, :, 512:1024])
            nc.sync.dma_start(xv_sb[:, 2:4], xv_in[:, 2:4])
            nc.sync.dma_start(xq_sb[:, :, 512:NQ], xq_in[:, :, 512:NQ])
            nc.sync.dma_start(kt_sb[:, :, 1024:1536], xk_in[:, :, 1024:1536])
            nc.sync.dma_start(xv_sb[:, 4:8], xv_in[:, 4:8])
            nc.sync.dma_start(kt_sb[:, :, 1536:2048], xk_in[:, :, 1536:2048])
            nc.sync.dma_start(xv_sb[:, 8:12], xv_in[:, 8:12])
            nc.sync.dma_start(xv_sb[:, 12:16], xv_in[:, 12:16])
            nc.sync.dma_start(wvh_sb[:, :], wvh_in[:, :])
            nc.sync.dma_start(wvl_sb[:, :], wvl_in[:, :])

            def qt_copy(ec, cols, ps):
                nc.scalar.activation(
                    out=qth_sb[:, ec, cols], in_=ps,
                    func=mybir.ActivationFunctionType.Copy, scale=s_qt,
                )
                nc.vector.scalar_tensor_tensor(
                    out=qtl_sb[:, ec, cols], in0=ps, scalar=s_qt,
                    in1=qth_sb[:, ec, cols], op0=MULT, op1=SUB,
                )

            # ============ G^T phase ============
            # qb=0 (q cols 0:512 = slots 0-3, consumed first) dc-pair-outer
            # so compute starts as soon as the first wqk/xq chunks land.
            # wq_sb chunk c = (colhalf a)*4 + dp holds [{hi,lo}, pair, 512].
            for a in range(2):  # column half == ec group
                ps_list = [
                    pp.tile([P, 512], F32, tag="ps", bufs=5, name=f"psg{a}_{i}")
                    for i in range(4)
                ]
                for dp in range(DCP):
                    for i in range(4):
                        co = slice(i * P, (i + 1) * P)
                        nc.tensor.matmul(
                            ps_list[i], wq_sb[:, a * 4 + dp, :, co],
                            xq_sb[:, 2 * dp:2 * dp + 2, 0:512],
                            start=(dp == 0), stop=(dp == DCP - 1),
                            perf_mode=DR,
                        )
                for i in range(4):
                    qt_copy(a * 4 + i, slice(0, 512), ps_list[i])
            # qb=1 (q cols 512:1024) ec-outer; emitted lazily after slot 3
            # so the early slots aren't gated on the qb=1 Xq DMA
            def emit_gqb1(ec0, ec1):
                for ec in range(ec0, ec1):
                    ps = pp.tile([P, 512], F32, tag="ps", bufs=5)
                    a, i = ec // 4, ec % 4
                    co = slice(i * P, (i + 1) * P)
                    for dp in range(DCP):
                        nc.tensor.matmul(
                            ps, wq_sb[:, a * 4 + dp, :, co],
                            xq_sb[:, 2 * dp:2 * dp + 2, 512:NQ],
                            start=(dp == 0), stop=(dp == DCP - 1),
                            perf_mode=DR,
                        )
                    qt_copy(ec, slice(512, NQ), ps)

            # ================= attention =================
            with tc.tile_pool(name="attn", bufs=2) as ap:
                slot_state = {}
                pending = []

                def consume(s, g):
                    st = slot_state[s]
                    nt = st["nt"]
                    cnt = min(4, nt - g * 4)
                    a16, ah, al = st["a16"], st["ah"], st["al"]
                    for i in range(cnt):
                        t = g * 4 + i
                        nc.tensor.matmul(
                            st["ps_den"], a16[:, t, :], ones_sb,
                            start=(t == 0), stop=(t == nt - 1),
                        )
                    if g != st["ng"] - 1:
                        return
                    # slot finished: U^T hi/lo; Y deferred to the end
                    nc.vector.reciprocal(
                        rec_sb[:, s:s + 1], st["ps_den"]
                    )
                    np_ = nt // 2
                    for dq in range(2):
                        ps_u = pp.tile(
                            [P, 512], F32, tag="ua", bufs=2,
                            name=f"psu{s}_{dq}",
                        )
                        for i in range(4):
                            dc = dq * 4 + i
                            ds = slice(dc * P, (dc + 1) * P)
                            po = slice(i * P, (i + 1) * P)
                            for kp in range(np_):
                                ks = slice(2 * kp, 2 * kp + 2)
                                nc.tensor.matmul(
                                    ps_u[:, po], xv_sb[:, ks, 0, ds],
                                    ah[:, ks, :],
                                    start=(kp == 0), stop=False, perf_mode=DR,
                                )
                                nc.tensor.matmul(
                                    ps_u[:, po], xv_sb[:, ks, 0, ds],
                                    al[:, ks, :],
                                    start=False, stop=False, perf_mode=DR,
                                )
                                nc.tensor.matmul(
                                    ps_u[:, po], xv_sb[:, ks, 1, ds],
                                    ah[:, ks, :],
                                    start=False, stop=(kp == np_ - 1),
                                    perf_mode=DR,
                                )
                        ucols = slice(dq * 4, dq * 4 + 4)
                        nc.scalar.activation(
                            out=uth_sb[:, s, ucols, :], in_=ps_u,
                            func=mybir.ActivationFunctionType.Copy,
                            scale=s_ut,
                        )
                        nc.vector.scalar_tensor_tensor(
                            out=utl_sb[:, s, ucols, :], in0=ps_u,
                            scalar=s_ut, in1=uth_sb[:, s, ucols, :],
                            op0=MULT, op1=SUB,
                        )
                    del slot_state[s]

                for s in range(NSLOT):
                    if s == 4:
                        emit_gqb1(0, 8)
                        while pending:
                            consume(*pending.pop(0))
                    L = 256 * (s + 1)
                    nt = L // P
                    ng = (nt + 3) // 4
                    slot_state[s] = {
                        "nt": nt,
                        "ng": ng,
                        "a16": ap.tile(
                            [P, S // P, P], BF16, tag="a16", bufs=3,
                            name=f"a16_{s}",
                        ),
                        "ah": ap.tile(
                            [P, S // P, P], F8, tag="ah", bufs=3,
                            name=f"ah_{s}",
                        ),
                        "al": ap.tile(
                            [P, S // P, P], F8, tag="al", bufs=3,
                            name=f"al_{s}",
                        ),
                        "ps_den": pp.tile(
                            [P, 1], F32, tag="psden", bufs=1, name=f"psden{s}"
                        ),
                    }
                    st = slot_state[s]
                    qs = slice(s * P, (s + 1) * P)
                    for g in range(ng):
                        cnt = min(4, nt - g * 4)
                        psT = pp.tile([P, 512], F32, tag="ps", bufs=5)
                        for i in range(cnt):
                            t = g * 4 + i
                            po = slice(i * P, (i + 1) * P)
                            ts = slice(t * P, (t + 1) * P)
                            for j in range(DCP):
                                sl = slice(2 * j, 2 * j + 2)
                                nc.tensor.matmul(
                                    psT[:, po], kt_sb[:, sl, ts],
                                    qth_sb[:, sl, qs],
                                    start=(j == 0), stop=False, perf_mode=DR,
                                )
                                nc.tensor.matmul(
                                    psT[:, po], kt_sb[:, sl, ts],
                                    qtl_sb[:, sl, qs],
                                    start=False, stop=(j == DCP - 1),
                                    perf_mode=DR,
                                )
                        if g == ng - 1:
                            nc.vector.tensor_add(
                                out=psT[:, (cnt - 2) * P:(cnt - 1) * P],
                                in0=psT[:, (cnt - 2) * P:(cnt - 1) * P],
                                in1=mask_a,
                            )
                            nc.vector.tensor_add(
                                out=psT[:, (cnt - 1) * P:cnt * P],
                                in0=psT[:, (cnt - 1) * P:cnt * P],
                                in1=mask_b,
                            )
                        gs = slice(g * 4, g * 4 + cnt)
                        nc.scalar.activation(
                            out=st["a16"][:, gs, :], in_=psT[:, :cnt * P],
                            func=mybir.ActivationFunctionType.Exp,
                            scale=s_exp,
                        )
                        nc.scalar.activation(
                            out=st["ah"][:, gs, :], in_=psT[:, :cnt * P],
                            func=mybir.ActivationFunctionType.Exp,
                            scale=s_exp,
                        )
                        nc.vector.tensor_sub(
                            out=st["al"][:, gs, :],
                            in0=st["a16"][:, gs, :],
                            in1=st["ah"][:, gs, :],
                        )
                        if len(pending) >= 2:
                            consume(*pending.pop(0))
                        pending.append((s, g))
                while pending:
                    consume(*pending.pop(0))

                # ---- deferred Y = U @ Wv for all slots ----
                for s in range(NSLOT):
                    out_sb = ap.tile([P, D], BF16, tag="out", bufs=4)
                    rec = rec_sb[:, s:s + 1]
                    for eq in range(4):  # 256-wide chains: copy+DMA start
                        es = slice(eq * 256, (eq + 1) * 256)  # mid-GEMM
                        ps_y = pp.tile(
                            [P, 256], F32, tag="ps", bufs=5,
                            name=f"psy{eq}_{s}",
                        )
                        for dp in range(DCP):
                            sl = slice(2 * dp, 2 * dp + 2)
                            nc.tensor.matmul(
                                ps_y, uth_sb[:, s, sl, :], wvh_sb[:, sl, es],
                                start=(dp == 0), stop=False, perf_mode=DR,
                            )
                            nc.tensor.matmul(
                                ps_y, uth_sb[:, s, sl, :], wvl_sb[:, sl, es],
                                start=False, stop=False, perf_mode=DR,
                            )
                            nc.tensor.matmul(
                                ps_y, utl_sb[:, s, sl, :], wvh_sb[:, sl, es],
                                start=False, stop=(dp == DCP - 1),
                                perf_mode=DR,
                            )
                        if eq % 2 == 0:
                            nc.scalar.activation(
                                out=out_sb[:, es], in_=ps_y,
                                func=mybir.ActivationFunctionType.Copy,
                                scale=rec,
                            )
                        else:
                            nc.vector.tensor_scalar_mul(
                                out_sb[:, es], ps_y, rec
                            )
                        eng = nc.gpsimd if eq % 2 == 0 else nc.sync
                        eng.dma_start(
                            out[s * P:(s + 1) * P, es], out_sb[:, es]
                        )

    nc.compile()
    return nc


_NC_CACHE = None


def _get_nc():
    global _NC_CACHE
    if _NC_CACHE is None:
        _NC_CACHE = build_attention_nc()
    return _NC_CACHE


def _make_masks(h):
    """Transposed masks [key kk, query r] for the last two key tiles."""
    import ml_dtypes

    kk = np.arange(P)[:, None]
    r = np.arange(P)[None, :]
    tri = np.where(kk <= r, 0.0, NEG).astype(np.float32)
    if h == 0:
        mask_a, mask_b = tri, np.full((P, P), NEG, dtype=np.float32)
    else:
        mask_a, mask_b = np.zeros((P, P), dtype=np.float32), tri
    return mask_a.astype(ml_dtypes.bfloat16), mask_b.astype(ml_dtypes.bfloat16)


def _pow2_floor(x):
    return float(2.0 ** np.floor(np.log2(x)))


def kernel(
    inputs_for_keys,
    inputs_for_values,
    inputs_for_queries,
    weight_K,
    weight_V,
    weight_Q,
    trace=False,
):
    import ml_dtypes

    f8 = ml_dtypes.float8_e4m3

    xk_full = np.asarray(inputs_for_keys, dtype=np.float32)
    xv_full = np.asarray(inputs_for_values, dtype=np.float32)
    xq_full = np.asarray(inputs_for_queries, dtype=np.float32)
    w_v = np.asarray(weight_V, dtype=np.float32)
    w_qk = (
        np.asarray(weight_Q, dtype=np.float32)
        @ np.asarray(weight_K, dtype=np.float32).T
    )

    # power-of-two scales (range only; fp8 rel precision is scale-free)
    aq = _pow2_floor(192.0 / max(np.abs(xq_full).max(), 1e-30))
    ak = _pow2_floor(192.0 / max(np.abs(xk_full).max(), 1e-30))
    av = _pow2_floor(192.0 / max(np.abs(xv_full).max(), 1e-30))
    aqk = _pow2_floor(192.0 / max(np.abs(w_qk).max(), 1e-30))
    aw = _pow2_floor(192.0 / max(np.abs(w_v).max(), 1e-30))
    # G row scale: G = Xq@Wqk is near-Gaussian; absmax ~<= 8*sigma
    sg = float(
        np.sqrt((w_qk ** 2).mean() * D * (xq_full ** 2).mean())
    )
    ag = _pow2_floor(192.0 / (8.0 * sg))
    # U = attn@Xv (unnormalized): sigma_U^2 ~= sum_k E[a^2] * sigma_v^2
    ss = SCALE * sg * float(np.sqrt((xk_full ** 2).mean() * D))
    ea2 = float(np.exp(2.0 * ss * ss))
    su = float(np.sqrt(S * ea2 * (xv_full ** 2).mean()))
    au = _pow2_floor(16.0 / su)

    s_exp = SCALE / (ag * ak)
    g_qt = ag / (aq * aqk)      # psum(G*aq*aqk) -> qt = G*ag
    g_ut = au / av              # psum(U*av) -> ut = U*au
    c_ones = au * aw            # den vector value; rec = 1/(c*den)

    def hl(x, scale, layout):
        xs = np.asarray(x, np.float32) * scale
        hi = xs.astype(f8)
        lo = (xs - hi.astype(np.float32)).astype(f8)
        return layout(hi), layout(lo)

    def _xT(x):  # [rows, D] -> [P, DC, rows] (d on partitions)
        xt = np.asarray(x).T.reshape(DC, P, x.shape[0])
        return np.ascontiguousarray(xt.transpose(1, 0, 2))

    def _w(w):  # [D, D] -> [P, DC, D]
        wr = np.asarray(w).reshape(DC, P, D)
        return np.ascontiguousarray(wr.transpose(1, 0, 2))

    def _xv(x):  # [S, D] -> [P, S//P, D] (keys on partitions)
        xr = np.asarray(x).reshape(S // P, P, D)
        return np.ascontiguousarray(xr.transpose(1, 0, 2))

    wqh = _w((w_qk * aqk).astype(f8))
    wvh, wvl = hl(w_v, aw, _w)
    # pack wqk chunks [P, 8, 2, 512]: chunk (a*4+dp) = row-pair x col half a
    wq_pack = np.empty((P, 8, 2, 512), dtype=f8)
    for a in range(2):
        for dp in range(DCP):
            cs = slice(a * 512, (a + 1) * 512)
            wq_pack[:, a * 4 + dp] = wqh[:, 2 * dp:2 * dp + 2, cs]
    xkT = [_xT((xk_full[b] * ak).astype(f8)) for b in range(B)]
    xvp = []
    for b in range(B):
        h_, l_ = hl(xv_full[b], av, _xv)
        xi = np.empty((P, S // P, 2, D), dtype=f8)
        xi[:, :, 0] = h_
        xi[:, :, 1] = l_
        xvp.append(xi)

    masks = [_make_masks(0), _make_masks(1)]
    ones_np = np.full((P, 1), c_ones, np.float32).astype(ml_dtypes.bfloat16)
    scl_np = np.zeros((P, 4), np.float32)
    scl_np[:, 0] = s_exp
    scl_np[:, 1] = g_qt
    scl_np[:, 2] = g_ut

    in_maps = []
    for c in range(2 * B):
        b, h = c // 2, c % 2
        rows = np.concatenate(
            [
                xq_full[b, 256 * s + 128 * h: 256 * s + 128 * h + P, :]
                for s in range(NSLOT)
            ],
            axis=0,
        )
        xqc = _xT((rows * aq).astype(f8))
        in_maps.append(
            {
                "xq": xqc,
                "xk": xkT[b],
                "xv": xvp[b],
                "wq": wq_pack,
                "wvh": wvh,
                "wvl": wvl,
                "mask_a": masks[h][0],
                "mask_b": masks[h][1],
                "ones": ones_np,
                "scl": scl_np,
            }
        )

    nc = _get_nc()
    res = bass_utils.run_bass_kernel_spmd(
        nc, in_maps, core_ids=list(range(2 * B)), trace=trace
    )

    out = np.empty((B, S, D), dtype=np.float32)
    for c in range(2 * B):
        b, h = c // 2, c % 2
        o = np.asarray(res.results[c]["out"], dtype=np.float32)
        for s in range(NSLOT):
            out[b, 256 * s + 128 * h: 256 * s + 128 * h + P, :] = o[
                s * P:(s + 1) * P, :
            ]

    if trace:
        return out, res
    return out


# revision 8
# speedup vs baseline: 1.0105x; 1.0105x over previous
"""Causal single-head attention (B=4, S=2048, D=1024) on 8 TRN2 NeuronCores.

fp8(e4m3) DoubleRow rewrite of the bf16 baseline.  Same sharding: core
c -> (batch b = c//2, half h = c%2); 8 query slots of 128 rows with padded
causal key-lengths L_s = 256*(s+1); scores computed transposed; Wqk = Wq@Wk^T
fused host-side; out = ((attn @ X_v) @ W_v) / den.

Quantization scheme (all matmuls fp8e4 DoubleRow, PSUM fp32):
  G = Xq @ Wqk      single-fp8 (Wqk hi only, Xq fp8); G -> qt hi+lo on
                    device (ACT copy + DVE scalar_tensor_tensor residual).
  scores = qt@K^T   2-product: qt hi+lo (device), K^T single fp8 (host).
  U = attn @ Xv     3-product: attn hi+lo (ACT exp f8 + bf16, DVE sub),
                    Xv hi+lo (host).  U -> ut hi+lo on device.
  Y = U @ Wv        3-product: ut hi+lo (device), Wv hi+lo (host).
hi+lo fp8 pairs represent bf16 values exactly, so the residual GEMMs have
~bf16-level error; the 2-product GEMMs err only by the single side's fp8
quantization.  The softmax denominator is summed from the quantized bf16
attn (== hi+lo exactly), cancelling common-mode quantization error; the
"ones" den vector carries value au*aw so 1/den absorbs all output scales.
Scales are powers of two shipped in a [P,4] f32 vector, so the compiled
program is input-independent.

Schedule: the cost model serializes all DMA on one shared resource
(~360 GB/s) with ~0.6us per-issue overhead, so all bulk input DMA goes on
ONE queue (sync) in exact first-consumption order; consts + Xq ride the
scalar queue; output DMA on the SWDGE.  Slots run ASCENDING (0..7) so K/V
chunks stream just-in-time, and the 8 Y GEMMs are deferred to the end
(ut hi/lo tiles are tiny and stay resident) where they keep the PE busy
through the tail while Wv has long arrived.

NOTE (hazard, empirical): interleaving start/stop matmul accumulation chains
across sub-regions of a single PSUM bank corrupts results on HW; keep each
sub-tile's chain contiguous (interleaving across banks is fine).
"""

import numpy as np

import concourse.bacc as bacc
import concourse.mybir as mybir
import concourse.tile as tile
from concourse import bass_utils

B, S, D = 4, 2048, 1024
P = 128
DC = D // P          # 8 contraction chunks
DCP = DC // 2        # 4 DoubleRow contraction pairs
EC = D // P
NSLOT = 8
NQ = NSLOT * P
SCALE = 1.0 / float(np.sqrt(np.float32(S)))
NEG = -1.0e9

F32 = mybir.dt.float32
BF16 = mybir.dt.bfloat16
F8 = mybir.dt.float8e4
DR = mybir.MatmulPerfMode.DoubleRow
MULT = mybir.AluOpType.mult
SUB = mybir.AluOpType.subtract


def build_attention_nc():
    nc = bacc.Bacc("TRN2", target_bir_lowering=False)

    xq_in = nc.dram_tensor("xq", [P, DC, NQ], F8, kind="ExternalInput")
    xk_in = nc.dram_tensor("xk", [P, EC, S], F8, kind="ExternalInput")
    xkl_in = nc.dram_tensor("xkl", [P, EC, S], F8, kind="ExternalInput")
    # X_v hi/lo interleaved per key tile: [p, tile, {hi,lo}, d]
    xv_in = nc.dram_tensor("xv", [P, S // P, 2, D], F8, kind="ExternalInput")
    # Wqk packed in consumption-ordered chunks: chunk c = (colhalf a, dcpair
    # dp) at c = a*4+dp, holding [{row0,row1}, 512 cols] (hi only; the G
    # GEMM runs single-fp8 -- Xq and Wqk each quantized once).
    wq_in = nc.dram_tensor("wq", [P, 8, 2, 512], F8, kind="ExternalInput")
    wvh_in = nc.dram_tensor("wvh", [P, DC, D], F8, kind="ExternalInput")
    wvl_in = nc.dram_tensor("wvl", [P, DC, D], F8, kind="ExternalInput")
    mask_a_in = nc.dram_tensor("mask_a", [P, P], BF16, kind="ExternalInput")
    mask_b_in = nc.dram_tensor("mask_b", [P, P], BF16, kind="ExternalInput")
    ones_in = nc.dram_tensor("ones", [P, 1], BF16, kind="ExternalInput")
    scl_in = nc.dram_tensor("scl", [P, 4], F32, kind="ExternalInput")
    out = nc.dram_tensor("out", [NQ, D], BF16, kind="ExternalOutput")

    with tile.TileContext(nc) as tc:
        with (
            tc.tile_pool(name="res", bufs=1) as res,
            tc.tile_pool(name="psum", bufs=4, space="PSUM") as pp,
        ):
            kt_sb = res.tile([P, EC, S], F8)        # K^T hi [e, keys]
            ktl_sb = res.tile([P, EC, S], F8)       # K^T lo (slots 4-7)
            xv_sb = res.tile([P, S // P, 2, D], F8)  # X_v hi/lo interleaved
            xq_sb = res.tile([P, DC, NQ], F8)       # X_q^T [d, q]
            qth_sb = res.tile([P, EC, NQ], F8)      # Q^T hi [e, q]
            qtl_sb = res.tile([P, EC, 512], F8)     # Q^T lo (qb0 only)
            wq_sb = res.tile([P, 8, 2, 512], F8)  # Wqk packed chunks
            wvh_sb = res.tile([P, DC, D], F8)
            wvl_sb = res.tile([P, DC, D], F8)
            uth_sb = res.tile([P, NSLOT, DC, P], F8)  # U^T hi (all slots)
            utl_sb = res.tile([P, NSLOT, DC, P], F8)
            rec_sb = res.tile([P, NSLOT], F32)        # 1/(c*den) per slot
            mask_a = res.tile([P, P], BF16)
            mask_b = res.tile([P, P], BF16)
            ones_sb = res.tile([P, 1], BF16)
            scl_sb = res.tile([P, 4], F32)
            s_exp = scl_sb[:, 0:1]   # exp scale = SCALE/(ag*ak)
            s_qt = scl_sb[:, 1:2]    # gamma_g: psum->qt scale
            s_ut = scl_sb[:, 2:3]    # gamma_u: psum->ut scale

            # ---- DMA: Xq-qb0 + consts on scalar; all bulk on sync, in
            # first-consumption order.  Each issue costs ~0.63us on its
            # queue, so the stream uses few, large chunks. ----
            nc.scalar.dma_start(xq_sb[:, 0:2, 0:512], xq_in[:, 0:2, 0:512])
            nc.gpsimd.dma_start(xq_sb[:, 2:8, 0:512], xq_in[:, 2:8, 0:512])
            nc.gpsimd.dma_start(scl_sb, scl_in[:, :])
            nc.gpsimd.dma_start(ones_sb, ones_in[:, :])
            nc.gpsimd.dma_start(mask_a, mask_a_in[:, :])
            nc.gpsimd.dma_start(mask_b, mask_b_in[:, :])
            nc.sync.dma_start(wq_sb[:, 0:1], wq_in[:, 0:1])
            nc.sync.dma_start(wq_sb[:, 1:4], wq_in[:, 1:4])
            nc.sync.dma_start(wq_sb[:, 4:8], wq_in[:, 4:8])
            nc.sync.dma_start(kt_sb[:, :, 0:512], xk_in[:, :, 0:512])
            nc.sync.dma_start(xv_sb[:, 0:2], xv_in[:, 0:2])
            nc.sync.dma_start(kt_sb[:, :, 512:1024], xk_in[:, :, 512:1024])
            nc.sync.dma_start(xv_sb[:, 2:4], xv_in[:, 2:4])
            nc.sync.dma_start(xq_sb[:, :, 512:NQ], xq_in[:, :, 512:NQ])
            nc.sync.dma_start(kt_sb[:, :, 1024:1536], xk_in[:, :, 1024:1536])
            nc.sync.dma_start(xv_sb[:, 4:8], xv_in[:, 4:8])
            nc.sync.dma_start(ktl_sb[:, :, 0:1024], xkl_in[:, :, 0:1024])
            nc.sync.dma_start(kt_sb[:, :, 1536:2048], xk_in[:, :, 1536:2048])
            nc.sync.dma_start(xv_sb[:, 8:12], xv_in[:, 8:12])
            nc.sync.dma_start(ktl_sb[:, :, 1024:2048], xkl_in[:, :, 1024:2048])
            nc.sync.dma_start(xv_sb[:, 12:16], xv_in[:, 12:16])
            nc.sync.dma_start(wvh_sb[:, :], wvh_in[:, :])
            nc.sync.dma_start(wvl_sb[:, :], wvl_in[:, :])

            def qt_copy(ec, cols, ps, lo=True):
                nc.scalar.activation(
                    out=qth_sb[:, ec, cols], in_=ps,
                    func=mybir.ActivationFunctionType.Copy, scale=s_qt,
                )
                if lo:
                    nc.vector.scalar_tensor_tensor(
                        out=qtl_sb[:, ec, cols], in0=ps, scalar=s_qt,
                        in1=qth_sb[:, ec, cols], op0=MULT, op1=SUB,
                    )

            # ============ G^T phase ============
            # qb=0 (q cols 0:512 = slots 0-3, consumed first) dc-pair-outer
            # so compute starts as soon as the first wqk/xq chunks land.
            # wq_sb chunk c = (colhalf a)*4 + dp holds [{hi,lo}, pair, 512].
            for a in range(2):  # column half == ec group
                ps_list = [
                    pp.tile([P, 512], F32, tag="ps", bufs=5, name=f"psg{a}_{i}")
                    for i in range(4)
                ]
                for dp in range(DCP):
                    for i in range(4):
                        co = slice(i * P, (i + 1) * P)
                        nc.tensor.matmul(
                            ps_list[i], wq_sb[:, a * 4 + dp, :, co],
                            xq_sb[:, 2 * dp:2 * dp + 2, 0:512],
                            start=(dp == 0), stop=(dp == DCP - 1),
                            perf_mode=DR,
                        )
                for i in range(4):
                    qt_copy(a * 4 + i, slice(0, 512), ps_list[i])
            # qb=1 (q cols 512:1024) ec-outer; emitted lazily after slot 3
            # so the early slots aren't gated on the qb=1 Xq DMA
            def emit_gqb1(ec0, ec1):
                for ec in range(ec0, ec1):
                    ps = pp.tile([P, 512], F32, tag="ps", bufs=5)
                    a, i = ec // 4, ec % 4
                    co = slice(i * P, (i + 1) * P)
                    for dp in range(DCP):
                        nc.tensor.matmul(
                            ps, wq_sb[:, a * 4 + dp, :, co],
                            xq_sb[:, 2 * dp:2 * dp + 2, 512:NQ],
                            start=(dp == 0), stop=(dp == DCP - 1),
                            perf_mode=DR,
                        )
                    qt_copy(ec, slice(512, NQ), ps, lo=False)

            # ================= attention =================
            with tc.tile_pool(name="attn", bufs=2) as ap:
                slot_state = {}
                pending = []

                def consume(s, g):
                    st = slot_state[s]
                    nt = st["nt"]
                    cnt = min(4, nt - g * 4)
                    a16, ah, al = st["a16"], st["ah"], st["al"]
                    for i in range(cnt):
                        t = g * 4 + i
                        nc.tensor.matmul(
                            st["ps_den"], a16[:, t, :], ones_sb,
                            start=(t == 0), stop=(t == nt - 1),
                        )
                    if g != st["ng"] - 1:
                        return
                    # slot finished: U^T hi/lo; Y deferred to the end
                    nc.vector.reciprocal(
                        rec_sb[:, s:s + 1], st["ps_den"]
                    )
                    np_ = nt // 2
                    for dq in range(2):
                        ps_u = pp.tile(
                            [P, 512], F32, tag="ua", bufs=2,
                            name=f"psu{s}_{dq}",
                        )
                        for i in range(4):
                            dc = dq * 4 + i
                            ds = slice(dc * P, (dc + 1) * P)
                            po = slice(i * P, (i + 1) * P)
                            for kp in range(np_):
                                ks = slice(2 * kp, 2 * kp + 2)
                                nc.tensor.matmul(
                                    ps_u[:, po], xv_sb[:, ks, 0, ds],
                                    ah[:, ks, :],
                                    start=(kp == 0), stop=False, perf_mode=DR,
                                )
                                nc.tensor.matmul(
                                    ps_u[:, po], xv_sb[:, ks, 0, ds],
                                    al[:, ks, :],
                                    start=False, stop=False, perf_mode=DR,
                                )
                                nc.tensor.matmul(
                                    ps_u[:, po], xv_sb[:, ks, 1, ds],
                                    ah[:, ks, :],
                                    start=False, stop=(kp == np_ - 1),
                                    perf_mode=DR,
                                )
                        ucols = slice(dq * 4, dq * 4 + 4)
                        nc.scalar.activation(
                            out=uth_sb[:, s, ucols, :], in_=ps_u,
                            func=mybir.ActivationFunctionType.Copy,
                            scale=s_ut,
                        )
                        nc.vector.scalar_tensor_tensor(
                            out=utl_sb[:, s, ucols, :], in0=ps_u,
                            scalar=s_ut, in1=uth_sb[:, s, ucols, :],
                            op0=MULT, op1=SUB,
                        )
                    del slot_state[s]

                for s in range(NSLOT):
                    if s == 4:
                        emit_gqb1(0, 8)
                        while pending:
                            consume(*pending.pop(0))
                    L = 256 * (s + 1)
                    nt = L // P
                    ng = (nt + 3) // 4
                    slot_state[s] = {
                        "nt": nt,
                        "ng": ng,
                        "a16": ap.tile(
                            [P, S // P, P], BF16, tag="a16", bufs=3,
                            name=f"a16_{s}",
                        ),
                        "ah": ap.tile(
                            [P, S // P, P], F8, tag="ah", bufs=3,
                            name=f"ah_{s}",
                        ),
                        "al": ap.tile(
                            [P, S // P, P], F8, tag="al", bufs=3,
                            name=f"al_{s}",
                        ),
                        "ps_den": pp.tile(
                            [P, 1], F32, tag="psden", bufs=1, name=f"psden{s}"
                        ),
                    }
                    st = slot_state[s]
                    qs = slice(s * P, (s + 1) * P)
                    for g in range(ng):
                        cnt = min(4, nt - g * 4)
                        psT = pp.tile([P, 512], F32, tag="ps", bufs=5)
                        for i in range(cnt):
                            t = g * 4 + i
                            po = slice(i * P, (i + 1) * P)
                            ts = slice(t * P, (t + 1) * P)
                            for j in range(DCP):
                                sl = slice(2 * j, 2 * j + 2)
                                nc.tensor.matmul(
                                    psT[:, po], kt_sb[:, sl, ts],
                                    qth_sb[:, sl, qs],
                                    start=(j == 0), stop=False, perf_mode=DR,
                                )
                                if s < 4:
                                    nc.tensor.matmul(
                                        psT[:, po], kt_sb[:, sl, ts],
                                        qtl_sb[:, sl, qs],
                                        start=False, stop=(j == DCP - 1),
                                        perf_mode=DR,
                                    )
                                else:
                                    nc.tensor.matmul(
                                        psT[:, po], ktl_sb[:, sl, ts],
                                        qth_sb[:, sl, qs],
                                        start=False, stop=(j == DCP - 1),
                                        perf_mode=DR,
                                    )
                        if g == ng - 1:
                            nc.vector.tensor_add(
                                out=psT[:, (cnt - 2) * P:(cnt - 1) * P],
                                in0=psT[:, (cnt - 2) * P:(cnt - 1) * P],
                                in1=mask_a,
                            )
                            nc.vector.tensor_add(
                                out=psT[:, (cnt - 1) * P:cnt * P],
                                in0=psT[:, (cnt - 1) * P:cnt * P],
                                in1=mask_b,
                            )
                        gs = slice(g * 4, g * 4 + cnt)
                        nc.scalar.activation(
                            out=st["a16"][:, gs, :], in_=psT[:, :cnt * P],
                            func=mybir.ActivationFunctionType.Exp,
                            scale=s_exp,
                        )
                        nc.scalar.activation(
                            out=st["ah"][:, gs, :], in_=psT[:, :cnt * P],
                            func=mybir.ActivationFunctionType.Exp,
                            scale=s_exp,
                        )
                        nc.vector.tensor_sub(
                            out=st["al"][:, gs, :],
                            in0=st["a16"][:, gs, :],
                            in1=st["ah"][:, gs, :],
                        )
                        if len(pending) >= 2:
                            consume(*pending.pop(0))
                        pending.append((s, g))
                while pending:
                    consume(*pending.pop(0))

                # ---- deferred Y = U @ Wv for all slots ----
                for s in range(NSLOT):
                    out_sb = ap.tile([P, D], BF16, tag="out", bufs=4)
                    rec = rec_sb[:, s:s + 1]
                    for eq in range(4):  # 256-wide chains: copy+DMA start
                        es = slice(eq * 256, (eq + 1) * 256)  # mid-GEMM
                        ps_y = pp.tile(
                            [P, 256], F32, tag="ps", bufs=5,
                            name=f"psy{eq}_{s}",
                        )
                        for dp in range(DCP):
                            sl = slice(2 * dp, 2 * dp + 2)
                            nc.tensor.matmul(
                                ps_y, uth_sb[:, s, sl, :], wvh_sb[:, sl, es],
                                start=(dp == 0), stop=False, perf_mode=DR,
                            )
                            nc.tensor.matmul(
                                ps_y, uth_sb[:, s, sl, :], wvl_sb[:, sl, es],
                                start=False, stop=False, perf_mode=DR,
                            )
                            nc.tensor.matmul(
                                ps_y, utl_sb[:, s, sl, :], wvh_sb[:, sl, es],
                                start=False, stop=(dp == DCP - 1),
                                perf_mode=DR,
                            )
                        if eq % 2 == 0:
                            nc.scalar.activation(
                                out=out_sb[:, es], in_=ps_y,
                                func=mybir.ActivationFunctionType.Copy,
                                scale=rec,
                            )
                        else:
                            nc.vector.tensor_scalar_mul(
                                out_sb[:, es], ps_y, rec
                            )
                        eng = nc.gpsimd if eq % 2 == 0 else nc.sync
                        eng.dma_start(
                            out[s * P:(s + 1) * P, es], out_sb[:, es]
                        )

    nc.compile()
    return nc


_NC_CACHE = None


def _get_nc():
    global _NC_CACHE
    if _NC_CACHE is None:
        _NC_CACHE = build_attention_nc()
    return _NC_CACHE


def _make_masks(h):
    """Transposed masks [key kk, query r] for the last two key tiles."""
    import ml_dtypes

    kk = np.arange(P)[:, None]
    r = np.arange(P)[None, :]
    tri = np.where(kk <= r, 0.0, NEG).astype(np.float32)
    if h == 0:
        mask_a, mask_b = tri, np.full((P, P), NEG, dtype=np.float32)
    else:
        mask_a, mask_b = np.zeros((P, P), dtype=np.float32), tri
    return mask_a.astype(ml_dtypes.bfloat16), mask_b.astype(ml_dtypes.bfloat16)


def _pow2_floor(x):
    return float(2.0 ** np.floor(np.log2(x)))


def kernel(
    inputs_for_keys,
    inputs_for_values,
    inputs_for_queries,
    weight_K,
    weight_V,
    weight_Q,
    trace=False,
):
    import ml_dtypes

    f8 = ml_dtypes.float8_e4m3

    xk_full = np.asarray(inputs_for_keys, dtype=np.float32)
    xv_full = np.asarray(inputs_for_values, dtype=np.float32)
    xq_full = np.asarray(inputs_for_queries, dtype=np.float32)
    w_v = np.asarray(weight_V, dtype=np.float32)
    w_qk = (
        np.asarray(weight_Q, dtype=np.float32)
        @ np.asarray(weight_K, dtype=np.float32).T
    )

    # power-of-two scales (range only; fp8 rel precision is scale-free)
    aq = _pow2_floor(192.0 / max(np.abs(xq_full).max(), 1e-30))
    ak = _pow2_floor(192.0 / max(np.abs(xk_full).max(), 1e-30))
    av = _pow2_floor(192.0 / max(np.abs(xv_full).max(), 1e-30))
    aqk = _pow2_floor(192.0 / max(np.abs(w_qk).max(), 1e-30))
    aw = _pow2_floor(192.0 / max(np.abs(w_v).max(), 1e-30))
    # G row scale: G = Xq@Wqk is near-Gaussian; absmax ~<= 8*sigma
    sg = float(
        np.sqrt((w_qk ** 2).mean() * D * (xq_full ** 2).mean())
    )
    ag = _pow2_floor(192.0 / (8.0 * sg))
    # U = attn@Xv (unnormalized): sigma_U^2 ~= sum_k E[a^2] * sigma_v^2
    ss = SCALE * sg * float(np.sqrt((xk_full ** 2).mean() * D))
    ea2 = float(np.exp(2.0 * ss * ss))
    su = float(np.sqrt(S * ea2 * (xv_full ** 2).mean()))
    au = _pow2_floor(16.0 / su)

    s_exp = SCALE / (ag * ak)
    g_qt = ag / (aq * aqk)      # psum(G*aq*aqk) -> qt = G*ag
    g_ut = au / av              # psum(U*av) -> ut = U*au
    c_ones = au * aw            # den vector value; rec = 1/(c*den)

    def hl(x, scale, layout):
        xs = np.asarray(x, np.float32) * scale
        hi = xs.astype(f8)
        lo = (xs - hi.astype(np.float32)).astype(f8)
        return layout(hi), layout(lo)

    def _xT(x):  # [rows, D] -> [P, DC, rows] (d on partitions)
        xt = np.asarray(x).T.reshape(DC, P, x.shape[0])
        return np.ascontiguousarray(xt.transpose(1, 0, 2))

    def _w(w):  # [D, D] -> [P, DC, D]
        wr = np.asarray(w).reshape(DC, P, D)
        return np.ascontiguousarray(wr.transpose(1, 0, 2))

    def _xv(x):  # [S, D] -> [P, S//P, D] (keys on partitions)
        xr = np.asarray(x).reshape(S // P, P, D)
        return np.ascontiguousarray(xr.transpose(1, 0, 2))

    wqh = _w((w_qk * aqk).astype(f8))
    wvh, wvl = hl(w_v, aw, _w)
    # pack wqk chunks [P, 8, 2, 512]: chunk (a*4+dp) = row-pair x col half a
    wq_pack = np.empty((P, 8, 2, 512), dtype=f8)
    for a in range(2):
        for dp in range(DCP):
            cs = slice(a * 512, (a + 1) * 512)
            wq_pack[:, a * 4 + dp] = wqh[:, 2 * dp:2 * dp + 2, cs]
    xkp = [hl(xk_full[b], ak, _xT) for b in range(B)]
    xvp = []
    for b in range(B):
        h_, l_ = hl(xv_full[b], av, _xv)
        xi = np.empty((P, S // P, 2, D), dtype=f8)
        xi[:, :, 0] = h_
        xi[:, :, 1] = l_
        xvp.append(xi)

    masks = [_make_masks(0), _make_masks(1)]
    ones_np = np.full((P, 1), c_ones, np.float32).astype(ml_dtypes.bfloat16)
    scl_np = np.zeros((P, 4), np.float32)
    scl_np[:, 0] = s_exp
    scl_np[:, 1] = g_qt
    scl_np[:, 2] = g_ut

    in_maps = []
    for c in range(2 * B):
        b, h = c // 2, c % 2
        rows = np.concatenate(
            [
                xq_full[b, 256 * s + 128 * h: 256 * s + 128 * h + P, :]
                for s in range(NSLOT)
            ],
            axis=0,
        )
        xqc = _xT((rows * aq).astype(f8))
        in_maps.append(
            {
                "xq": xqc,
                "xk": xkp[b][0],
                "xkl": xkp[b][1],
                "xv": xvp[b],
                "wq": wq_pack,
                "wvh": wvh,
                "wvl": wvl,
                "mask_a": masks[h][0],
                "mask_b": masks[h][1],
                "ones": ones_np,
                "scl": scl_np,
            }
        )

    nc = _get_nc()
    res = bass_utils.run_bass_kernel_spmd(
        nc, in_maps, core_ids=list(range(2 * B)), trace=trace
    )

    out = np.empty((B, S, D), dtype=np.float32)
    for c in range(2 * B):
        b, h = c // 2, c % 2
        o = np.asarray(res.results[c]["out"], dtype=np.float32)
        for s in range(NSLOT):
            out[b, 256 * s + 128 * h: 256 * s + 128 * h + P, :] = o[
                s * P:(s + 1) * P, :
            ]

    if trace:
        return out, res
    return out


# revision 9
# speedup vs baseline: 1.0146x; 1.0041x over previous
"""Causal single-head attention (B=4, S=2048, D=1024) on 8 TRN2 NeuronCores.

fp8(e4m3) DoubleRow rewrite of the bf16 baseline.  Same sharding: core
c -> (batch b = c//2, half h = c%2); 8 query slots of 128 rows with padded
causal key-lengths L_s = 256*(s+1); scores computed transposed; Wqk = Wq@Wk^T
fused host-side; out = ((attn @ X_v) @ W_v) / den.

Quantization scheme (all matmuls fp8e4 DoubleRow, PSUM fp32):
  G = Xq @ Wqk      single-fp8 (Wqk hi only, Xq fp8); G -> qt hi+lo on
                    device (ACT copy + DVE scalar_tensor_tensor residual).
  scores = qt@K^T   2-product: qt hi+lo (device), K^T single fp8 (host).
  U = attn @ Xv     3-product: attn hi+lo (ACT exp f8 + bf16, DVE sub),
                    Xv hi+lo (host).  U -> ut hi+lo on device.
  Y = U @ Wv        3-product: ut hi+lo (device), Wv hi+lo (host).
hi+lo fp8 pairs represent bf16 values exactly, so the residual GEMMs have
~bf16-level error; the 2-product GEMMs err only by the single side's fp8
quantization.  The softmax denominator is summed from the quantized bf16
attn (== hi+lo exactly), cancelling common-mode quantization error; the
"ones" den vector carries value au*aw so 1/den absorbs all output scales.
Scales are powers of two shipped in a [P,4] f32 vector, so the compiled
program is input-independent.

Schedule: the cost model serializes all DMA on one shared resource
(~360 GB/s) with ~0.6us per-issue overhead, so all bulk input DMA goes on
ONE queue (sync) in exact first-consumption order; consts + Xq ride the
scalar queue; output DMA on the SWDGE.  Slots run ASCENDING (0..7) so K/V
chunks stream just-in-time, and the 8 Y GEMMs are deferred to the end
(ut hi/lo tiles are tiny and stay resident) where they keep the PE busy
through the tail while Wv has long arrived.

NOTE (hazard, empirical): interleaving start/stop matmul accumulation chains
across sub-regions of a single PSUM bank corrupts results on HW; keep each
sub-tile's chain contiguous (interleaving across banks is fine).
"""

import numpy as np

import concourse.bacc as bacc
import concourse.mybir as mybir
import concourse.tile as tile
from concourse import bass_utils

B, S, D = 4, 2048, 1024
P = 128
DC = D // P          # 8 contraction chunks
DCP = DC // 2        # 4 DoubleRow contraction pairs
EC = D // P
NSLOT = 8
NQ = NSLOT * P
SCALE = 1.0 / float(np.sqrt(np.float32(S)))
NEG = -1.0e9

F32 = mybir.dt.float32
BF16 = mybir.dt.bfloat16
F8 = mybir.dt.float8e4
DR = mybir.MatmulPerfMode.DoubleRow
MULT = mybir.AluOpType.mult
SUB = mybir.AluOpType.subtract


def build_attention_nc():
    nc = bacc.Bacc("TRN2", target_bir_lowering=False)

    xq_in = nc.dram_tensor("xq", [P, DC, NQ], F8, kind="ExternalInput")
    xk_in = nc.dram_tensor("xk", [P, EC, S], F8, kind="ExternalInput")
    xkl_in = nc.dram_tensor("xkl", [P, EC, S], F8, kind="ExternalInput")
    # X_v hi/lo interleaved per key tile: [p, tile, {hi,lo}, d]
    xv_in = nc.dram_tensor("xv", [P, S // P, 2, D], F8, kind="ExternalInput")
    # Wqk packed in consumption-ordered chunks: chunk c = (colhalf a, dcpair
    # dp) at c = a*4+dp, holding [{row0,row1}, 512 cols] (hi only; the G
    # GEMM runs single-fp8 -- Xq and Wqk each quantized once).
    wq_in = nc.dram_tensor("wq", [P, 8, 2, 512], F8, kind="ExternalInput")
    wvh_in = nc.dram_tensor("wvh", [P, DC, D], F8, kind="ExternalInput")
    wvl_in = nc.dram_tensor("wvl", [P, DC, D], F8, kind="ExternalInput")
    mask_a_in = nc.dram_tensor("mask_a", [P, P], BF16, kind="ExternalInput")
    mask_b_in = nc.dram_tensor("mask_b", [P, P], BF16, kind="ExternalInput")
    ones_in = nc.dram_tensor("ones", [P, 1], BF16, kind="ExternalInput")
    scl_in = nc.dram_tensor("scl", [P, 4], F32, kind="ExternalInput")
    out = nc.dram_tensor("out", [NQ, D], BF16, kind="ExternalOutput")

    with tile.TileContext(nc) as tc:
        with (
            tc.tile_pool(name="res", bufs=1) as res,
            tc.tile_pool(name="psum", bufs=4, space="PSUM") as pp,
        ):
            kt_sb = res.tile([P, EC, S], F8)        # K^T hi [e, keys]
            ktl_sb = res.tile([P, EC, S], F8)       # K^T lo (slots 4-7)
            xv_sb = res.tile([P, S // P, 2, D], F8)  # X_v hi/lo interleaved
            xq_sb = res.tile([P, DC, NQ], F8)       # X_q^T [d, q]
            qth_sb = res.tile([P, EC, NQ], F8)      # Q^T hi [e, q]
            qtl_sb = res.tile([P, EC, 512], F8)     # Q^T lo (qb0 only)
            wq_sb = res.tile([P, 8, 2, 512], F8)  # Wqk packed chunks
            wvh_sb = res.tile([P, DC, D], F8)
            wvl_sb = res.tile([P, DC, D], F8)
            uth_sb = res.tile([P, NSLOT, DC, P], F8)  # U^T hi (all slots)
            utl_sb = res.tile([P, NSLOT, DC, P], F8)
            rec_sb = res.tile([P, NSLOT], F32)        # 1/(c*den) per slot
            mask_a = res.tile([P, P], BF16)
            mask_b = res.tile([P, P], BF16)
            ones_sb = res.tile([P, 1], BF16)
            scl_sb = res.tile([P, 4], F32)
            s_exp = scl_sb[:, 0:1]   # exp scale = SCALE/(ag*ak)
            s_qt = scl_sb[:, 1:2]    # gamma_g: psum->qt scale
            s_ut = scl_sb[:, 2:3]    # gamma_u: psum->ut scale

            # ---- DMA: Xq-qb0 + consts on scalar; all bulk on sync, in
            # first-consumption order.  Each issue costs ~0.63us on its
            # queue, so the stream uses few, large chunks. ----
            nc.scalar.dma_start(xq_sb[:, 0:2, 0:512], xq_in[:, 0:2, 0:512])
            nc.gpsimd.dma_start(xq_sb[:, 2:8, 0:512], xq_in[:, 2:8, 0:512])
            nc.gpsimd.dma_start(scl_sb, scl_in[:, :])
            nc.gpsimd.dma_start(ones_sb, ones_in[:, :])
            nc.gpsimd.dma_start(mask_a, mask_a_in[:, :])
            nc.gpsimd.dma_start(mask_b, mask_b_in[:, :])
            nc.sync.dma_start(wq_sb[:, 0:1], wq_in[:, 0:1])
            nc.sync.dma_start(wq_sb[:, 1:4], wq_in[:, 1:4])
            nc.sync.dma_start(wq_sb[:, 4:8], wq_in[:, 4:8])
            nc.sync.dma_start(kt_sb[:, :, 0:512], xk_in[:, :, 0:512])
            nc.sync.dma_start(xv_sb[:, 0:2], xv_in[:, 0:2])
            nc.sync.dma_start(kt_sb[:, :, 512:1024], xk_in[:, :, 512:1024])
            nc.sync.dma_start(xv_sb[:, 2:4], xv_in[:, 2:4])
            nc.sync.dma_start(xq_sb[:, :, 512:NQ], xq_in[:, :, 512:NQ])
            nc.sync.dma_start(kt_sb[:, :, 1024:1536], xk_in[:, :, 1024:1536])
            nc.sync.dma_start(xv_sb[:, 4:8], xv_in[:, 4:8])
            nc.sync.dma_start(ktl_sb[:, :, 0:1024], xkl_in[:, :, 0:1024])
            nc.sync.dma_start(kt_sb[:, :, 1536:2048], xk_in[:, :, 1536:2048])
            nc.sync.dma_start(xv_sb[:, 8:12], xv_in[:, 8:12])
            nc.sync.dma_start(ktl_sb[:, :, 1024:2048], xkl_in[:, :, 1024:2048])
            nc.sync.dma_start(xv_sb[:, 12:16], xv_in[:, 12:16])
            nc.sync.dma_start(wvh_sb[:, :], wvh_in[:, :])
            nc.sync.dma_start(wvl_sb[:, :], wvl_in[:, :])

            def qt_copy(ec, cols, ps, lo=True, dve_hi=False):
                if dve_hi:
                    nc.vector.tensor_scalar_mul(
                        qth_sb[:, ec, cols], ps, s_qt
                    )
                else:
                    nc.scalar.activation(
                        out=qth_sb[:, ec, cols], in_=ps,
                        func=mybir.ActivationFunctionType.Copy, scale=s_qt,
                    )
                if lo:
                    nc.vector.scalar_tensor_tensor(
                        out=qtl_sb[:, ec, cols], in0=ps, scalar=s_qt,
                        in1=qth_sb[:, ec, cols], op0=MULT, op1=SUB,
                    )

            # ============ G^T phase ============
            # qb=0 (q cols 0:512 = slots 0-3, consumed first) dc-pair-outer
            # so compute starts as soon as the first wqk/xq chunks land.
            # wq_sb chunk c = (colhalf a)*4 + dp holds [{hi,lo}, pair, 512].
            for a in range(2):  # column half == ec group
                ps_list = [
                    pp.tile([P, 512], F32, tag="ps", bufs=5, name=f"psg{a}_{i}")
                    for i in range(4)
                ]
                for dp in range(DCP):
                    for i in range(4):
                        co = slice(i * P, (i + 1) * P)
                        nc.tensor.matmul(
                            ps_list[i], wq_sb[:, a * 4 + dp, :, co],
                            xq_sb[:, 2 * dp:2 * dp + 2, 0:512],
                            start=(dp == 0), stop=(dp == DCP - 1),
                            perf_mode=DR,
                        )
                for i in range(4):
                    qt_copy(a * 4 + i, slice(0, 512), ps_list[i])
            # qb=1 (q cols 512:1024) ec-outer; emitted lazily after slot 3
            # so the early slots aren't gated on the qb=1 Xq DMA
            def emit_gqb1(ec0, ec1):
                for ec in range(ec0, ec1):
                    ps = pp.tile([P, 512], F32, tag="ps", bufs=5)
                    a, i = ec // 4, ec % 4
                    co = slice(i * P, (i + 1) * P)
                    for dp in range(DCP):
                        nc.tensor.matmul(
                            ps, wq_sb[:, a * 4 + dp, :, co],
                            xq_sb[:, 2 * dp:2 * dp + 2, 512:NQ],
                            start=(dp == 0), stop=(dp == DCP - 1),
                            perf_mode=DR,
                        )
                    qt_copy(ec, slice(512, NQ), ps, lo=False,
                            dve_hi=(ec % 2 == 1))

            # ================= attention =================
            with tc.tile_pool(name="attn", bufs=2) as ap:
                slot_state = {}
                pending = []

                def consume(s, g):
                    st = slot_state[s]
                    nt = st["nt"]
                    cnt = min(4, nt - g * 4)
                    a16, ah, al = st["a16"], st["ah"], st["al"]
                    for i in range(cnt):
                        t = g * 4 + i
                        nc.tensor.matmul(
                            st["ps_den"], a16[:, t, :], ones_sb,
                            start=(t == 0), stop=(t == nt - 1),
                        )
                    if g != st["ng"] - 1:
                        return
                    # slot finished: U^T hi/lo; Y deferred to the end
                    nc.vector.reciprocal(
                        rec_sb[:, s:s + 1], st["ps_den"]
                    )
                    np_ = nt // 2
                    for dq in range(2):
                        ps_u = pp.tile(
                            [P, 512], F32, tag="ua", bufs=2,
                            name=f"psu{s}_{dq}",
                        )
                        for i in range(4):
                            dc = dq * 4 + i
                            ds = slice(dc * P, (dc + 1) * P)
                            po = slice(i * P, (i + 1) * P)
                            for kp in range(np_):
                                ks = slice(2 * kp, 2 * kp + 2)
                                nc.tensor.matmul(
                                    ps_u[:, po], xv_sb[:, ks, 0, ds],
                                    ah[:, ks, :],
                                    start=(kp == 0), stop=False, perf_mode=DR,
                                )
                                nc.tensor.matmul(
                                    ps_u[:, po], xv_sb[:, ks, 0, ds],
                                    al[:, ks, :],
                                    start=False, stop=False, perf_mode=DR,
                                )
                                nc.tensor.matmul(
                                    ps_u[:, po], xv_sb[:, ks, 1, ds],
                                    ah[:, ks, :],
                                    start=False, stop=(kp == np_ - 1),
                                    perf_mode=DR,
                                )
                        ucols = slice(dq * 4, dq * 4 + 4)
                        nc.scalar.activation(
                            out=uth_sb[:, s, ucols, :], in_=ps_u,
                            func=mybir.ActivationFunctionType.Copy,
                            scale=s_ut,
                        )
                        nc.vector.scalar_tensor_tensor(
                            out=utl_sb[:, s, ucols, :], in0=ps_u,
                            scalar=s_ut, in1=uth_sb[:, s, ucols, :],
                            op0=MULT, op1=SUB,
                        )
                    del slot_state[s]

                for s in range(NSLOT):
                    if s == 4:
                        emit_gqb1(0, 8)
                        while pending:
                            consume(*pending.pop(0))
                    L = 256 * (s + 1)
                    nt = L // P
                    ng = (nt + 3) // 4
                    slot_state[s] = {
                        "nt": nt,
                        "ng": ng,
                        "a16": ap.tile(
                            [P, S // P, P], BF16, tag="a16", bufs=3,
                            name=f"a16_{s}",
                        ),
                        "ah": ap.tile(
                            [P, S // P, P], F8, tag="ah", bufs=3,
                            name=f"ah_{s}",
                        ),
                        "al": ap.tile(
                            [P, S // P, P], F8, tag="al", bufs=3,
                            name=f"al_{s}",
                        ),
                        "ps_den": pp.tile(
                            [P, 1], F32, tag="psden", bufs=1, name=f"psden{s}"
                        ),
                    }
                    st = slot_state[s]
                    qs = slice(s * P, (s + 1) * P)
                    for g in range(ng):
                        cnt = min(4, nt - g * 4)
                        psT = pp.tile([P, 512], F32, tag="ps", bufs=5)
                        for i in range(cnt):
                            t = g * 4 + i
                            po = slice(i * P, (i + 1) * P)
                            ts = slice(t * P, (t + 1) * P)
                            for j in range(DCP):
                                sl = slice(2 * j, 2 * j + 2)
                                nc.tensor.matmul(
                                    psT[:, po], kt_sb[:, sl, ts],
                                    qth_sb[:, sl, qs],
                                    start=(j == 0), stop=False, perf_mode=DR,
                                )
                                if s < 4:
                                    nc.tensor.matmul(
                                        psT[:, po], kt_sb[:, sl, ts],
                                        qtl_sb[:, sl, qs],
                                        start=False, stop=(j == DCP - 1),
                                        perf_mode=DR,
                                    )
                                else:
                                    nc.tensor.matmul(
                                        psT[:, po], ktl_sb[:, sl, ts],
                                        qth_sb[:, sl, qs],
                                        start=False, stop=(j == DCP - 1),
                                        perf_mode=DR,
                                    )
                        if g == ng - 1:
                            nc.vector.tensor_add(
                                out=psT[:, (cnt - 2) * P:(cnt - 1) * P],
                                in0=psT[:, (cnt - 2) * P:(cnt - 1) * P],
                                in1=mask_a,
                            )
                            nc.vector.tensor_add(
                                out=psT[:, (cnt - 1) * P:cnt * P],
                                in0=psT[:, (cnt - 1) * P:cnt * P],
                                in1=mask_b,
                            )
                        gs = slice(g * 4, g * 4 + cnt)
                        nc.scalar.activation(
                            out=st["a16"][:, gs, :], in_=psT[:, :cnt * P],
                            func=mybir.ActivationFunctionType.Exp,
                            scale=s_exp,
                        )
                        nc.scalar.activation(
                            out=st["ah"][:, gs, :], in_=psT[:, :cnt * P],
                            func=mybir.ActivationFunctionType.Exp,
                            scale=s_exp,
                        )
                        nc.vector.tensor_sub(
                            out=st["al"][:, gs, :],
                            in0=st["a16"][:, gs, :],
                            in1=st["ah"][:, gs, :],
                        )
                        if len(pending) >= 2:
                            consume(*pending.pop(0))
                        pending.append((s, g))
                while pending:
                    consume(*pending.pop(0))

                # ---- deferred Y = U @ Wv for all slots ----
                for s in range(NSLOT):
                    out_sb = ap.tile([P, D], BF16, tag="out", bufs=4)
                    rec = rec_sb[:, s:s + 1]
                    for eq in range(4):  # 256-wide chains: copy+DMA start
                        es = slice(eq * 256, (eq + 1) * 256)  # mid-GEMM
                        ps_y = pp.tile(
                            [P, 256], F32, tag="ps", bufs=5,
                            name=f"psy{eq}_{s}",
                        )
                        for dp in range(DCP):
                            sl = slice(2 * dp, 2 * dp + 2)
                            nc.tensor.matmul(
                                ps_y, uth_sb[:, s, sl, :], wvh_sb[:, sl, es],
                                start=(dp == 0), stop=False, perf_mode=DR,
                            )
                            nc.tensor.matmul(
                                ps_y, uth_sb[:, s, sl, :], wvl_sb[:, sl, es],
                                start=False, stop=False, perf_mode=DR,
                            )
                            nc.tensor.matmul(
                                ps_y, utl_sb[:, s, sl, :], wvh_sb[:, sl, es],
                                start=False, stop=(dp == DCP - 1),
                                perf_mode=DR,
                            )
                        if eq % 2 == 0:
                            nc.scalar.activation(
                                out=out_sb[:, es], in_=ps_y,
                                func=mybir.ActivationFunctionType.Copy,
                                scale=rec,
                            )
                        else:
                            nc.vector.tensor_scalar_mul(
                                out_sb[:, es], ps_y, rec
                            )
                        eng = nc.gpsimd if eq % 2 == 0 else nc.sync
                        eng.dma_start(
                            out[s * P:(s + 1) * P, es], out_sb[:, es]
                        )

    nc.compile()
    return nc


_NC_CACHE = None


def _get_nc():
    global _NC_CACHE
    if _NC_CACHE is None:
        _NC_CACHE = build_attention_nc()
    return _NC_CACHE


def _make_masks(h):
    """Transposed masks [key kk, query r] for the last two key tiles."""
    import ml_dtypes

    kk = np.arange(P)[:, None]
    r = np.arange(P)[None, :]
    tri = np.where(kk <= r, 0.0, NEG).astype(np.float32)
    if h == 0:
        mask_a, mask_b = tri, np.full((P, P), NEG, dtype=np.float32)
    else:
        mask_a, mask_b = np.zeros((P, P), dtype=np.float32), tri
    return mask_a.astype(ml_dtypes.bfloat16), mask_b.astype(ml_dtypes.bfloat16)


def _pow2_floor(x):
    return float(2.0 ** np.floor(np.log2(x)))


def kernel(
    inputs_for_keys,
    inputs_for_values,
    inputs_for_queries,
    weight_K,
    weight_V,
    weight_Q,
    trace=False,
):
    import ml_dtypes

    f8 = ml_dtypes.float8_e4m3

    xk_full = np.asarray(inputs_for_keys, dtype=np.float32)
    xv_full = np.asarray(inputs_for_values, dtype=np.float32)
    xq_full = np.asarray(inputs_for_queries, dtype=np.float32)
    w_v = np.asarray(weight_V, dtype=np.float32)
    w_qk = (
        np.asarray(weight_Q, dtype=np.float32)
        @ np.asarray(weight_K, dtype=np.float32).T
    )

    # power-of-two scales (range only; fp8 rel precision is scale-free)
    aq = _pow2_floor(192.0 / max(np.abs(xq_full).max(), 1e-30))
    ak = _pow2_floor(192.0 / max(np.abs(xk_full).max(), 1e-30))
    av = _pow2_floor(192.0 / max(np.abs(xv_full).max(), 1e-30))
    aqk = _pow2_floor(192.0 / max(np.abs(w_qk).max(), 1e-30))
    aw = _pow2_floor(192.0 / max(np.abs(w_v).max(), 1e-30))
    # G row scale: G = Xq@Wqk is near-Gaussian; absmax ~<= 8*sigma
    sg = float(
        np.sqrt((w_qk ** 2).mean() * D * (xq_full ** 2).mean())
    )
    ag = _pow2_floor(192.0 / (8.0 * sg))
    # U = attn@Xv (unnormalized): sigma_U^2 ~= sum_k E[a^2] * sigma_v^2
    ss = SCALE * sg * float(np.sqrt((xk_full ** 2).mean() * D))
    ea2 = float(np.exp(2.0 * ss * ss))
    su = float(np.sqrt(S * ea2 * (xv_full ** 2).mean()))
    au = _pow2_floor(16.0 / su)

    s_exp = SCALE / (ag * ak)
    g_qt = ag / (aq * aqk)      # psum(G*aq*aqk) -> qt = G*ag
    g_ut = au / av              # psum(U*av) -> ut = U*au
    c_ones = au * aw            # den vector value; rec = 1/(c*den)

    def hl(x, scale, layout):
        xs = np.asarray(x, np.float32) * scale
        hi = xs.astype(f8)
        lo = (xs - hi.astype(np.float32)).astype(f8)
        return layout(hi), layout(lo)

    def _xT(x):  # [rows, D] -> [P, DC, rows] (d on partitions)
        xt = np.asarray(x).T.reshape(DC, P, x.shape[0])
        return np.ascontiguousarray(xt.transpose(1, 0, 2))

    def _w(w):  # [D, D] -> [P, DC, D]
        wr = np.asarray(w).reshape(DC, P, D)
        return np.ascontiguousarray(wr.transpose(1, 0, 2))

    def _xv(x):  # [S, D] -> [P, S//P, D] (keys on partitions)
        xr = np.asarray(x).reshape(S // P, P, D)
        return np.ascontiguousarray(xr.transpose(1, 0, 2))

    wqh = _w((w_qk * aqk).astype(f8))
    wvh, wvl = hl(w_v, aw, _w)
    # pack wqk chunks [P, 8, 2, 512]: chunk (a*4+dp) = row-pair x col half a
    wq_pack = np.empty((P, 8, 2, 512), dtype=f8)
    for a in range(2):
        for dp in range(DCP):
            cs = slice(a * 512, (a + 1) * 512)
            wq_pack[:, a * 4 + dp] = wqh[:, 2 * dp:2 * dp + 2, cs]
    xkp = [hl(xk_full[b], ak, _xT) for b in range(B)]
    xvp = []
    for b in range(B):
        h_, l_ = hl(xv_full[b], av, _xv)
        xi = np.empty((P, S // P, 2, D), dtype=f8)
        xi[:, :, 0] = h_
        xi[:, :, 1] = l_
        xvp.append(xi)

    masks = [_make_masks(0), _make_masks(1)]
    ones_np = np.full((P, 1), c_ones, np.float32).astype(ml_dtypes.bfloat16)
    scl_np = np.zeros((P, 4), np.float32)
    scl_np[:, 0] = s_exp
    scl_np[:, 1] = g_qt
    scl_np[:, 2] = g_ut

    in_maps = []
    for c in range(2 * B):
        b, h = c // 2, c % 2
        rows = np.concatenate(
            [
                xq_full[b, 256 * s + 128 * h: 256 * s + 128 * h + P, :]
                for s in range(NSLOT)
            ],
            axis=0,
        )
        xqc = _xT((rows * aq).astype(f8))
        in_maps.append(
            {
                "xq": xqc,
                "xk": xkp[b][0],
                "xkl": xkp[b][1],
                "xv": xvp[b],
                "wq": wq_pack,
                "wvh": wvh,
                "wvl": wvl,
                "mask_a": masks[h][0],
                "mask_b": masks[h][1],
                "ones": ones_np,
                "scl": scl_np,
            }
        )

    nc = _get_nc()
    res = bass_utils.run_bass_kernel_spmd(
        nc, in_maps, core_ids=list(range(2 * B)), trace=trace
    )

    out = np.empty((B, S, D), dtype=np.float32)
    for c in range(2 * B):
        b, h = c // 2, c % 2
        o = np.asarray(res.results[c]["out"], dtype=np.float32)
        for s in range(NSLOT):
            out[b, 256 * s + 128 * h: 256 * s + 128 * h + P, :] = o[
                s * P:(s + 1) * P, :
            ]

    if trace:
        return out, res
    return out


# revision 10
# speedup vs baseline: 1.0238x; 1.0091x over previous
"""Causal single-head attention (B=4, S=2048, D=1024) on 8 TRN2 NeuronCores.

fp8(e4m3) DoubleRow rewrite of the bf16 baseline.  Same sharding: core
c -> (batch b = c//2, half h = c%2); 8 query slots of 128 rows with padded
causal key-lengths L_s = 256*(s+1); scores computed transposed; Wqk = Wq@Wk^T
fused host-side; out = ((attn @ X_v) @ W_v) / den.

Quantization scheme (all matmuls fp8e4 DoubleRow, PSUM fp32):
  G = Xq @ Wqk      single-fp8 (Wqk hi only, Xq fp8); G -> qt hi+lo on
                    device (ACT copy + DVE scalar_tensor_tensor residual).
  scores = qt@K^T   2-product: qt hi+lo (device), K^T single fp8 (host).
  U = attn @ Xv     3-product: attn hi+lo (ACT exp f8 + bf16, DVE sub),
                    Xv hi+lo (host).  U -> ut hi+lo on device.
  Y = U @ Wv        3-product: ut hi+lo (device), Wv hi+lo (host).
hi+lo fp8 pairs represent bf16 values exactly, so the residual GEMMs have
~bf16-level error; the 2-product GEMMs err only by the single side's fp8
quantization.  The softmax denominator is summed from the quantized bf16
attn (== hi+lo exactly), cancelling common-mode quantization error; the
"ones" den vector carries value au*aw so 1/den absorbs all output scales.
Scales are powers of two shipped in a [P,4] f32 vector, so the compiled
program is input-independent.

Schedule: the cost model serializes all DMA on one shared resource
(~360 GB/s) with ~0.6us per-issue overhead, so all bulk input DMA goes on
ONE queue (sync) in exact first-consumption order; consts + Xq ride the
scalar queue; output DMA on the SWDGE.  Slots run ASCENDING (0..7) so K/V
chunks stream just-in-time, and the 8 Y GEMMs are deferred to the end
(ut hi/lo tiles are tiny and stay resident) where they keep the PE busy
through the tail while Wv has long arrived.

NOTE (hazard, empirical): interleaving start/stop matmul accumulation chains
across sub-regions of a single PSUM bank corrupts results on HW; keep each
sub-tile's chain contiguous (interleaving across banks is fine).
"""

import numpy as np

import concourse.bacc as bacc
import concourse.mybir as mybir
import concourse.tile as tile
from concourse import bass_utils

B, S, D = 4, 2048, 1024
P = 128
DC = D // P          # 8 contraction chunks
DCP = DC // 2        # 4 DoubleRow contraction pairs
EC = D // P
NSLOT = 8
NQ = NSLOT * P
SCALE = 1.0 / float(np.sqrt(np.float32(S)))
NEG = -1.0e9

F32 = mybir.dt.float32
BF16 = mybir.dt.bfloat16
F8 = mybir.dt.float8e4
DR = mybir.MatmulPerfMode.DoubleRow
MULT = mybir.AluOpType.mult
SUB = mybir.AluOpType.subtract


def build_attention_nc():
    nc = bacc.Bacc("TRN2", target_bir_lowering=False)

    xq_in = nc.dram_tensor("xq", [P, DC, NQ], F8, kind="ExternalInput")
    xk_in = nc.dram_tensor("xk", [P, EC, S], F8, kind="ExternalInput")
    xkl_in = nc.dram_tensor("xkl", [P, EC, S], F8, kind="ExternalInput")
    # X_v hi/lo interleaved per key tile: [p, tile, {hi,lo}, d]
    xv_in = nc.dram_tensor("xv", [P, S // P, 2, D], F8, kind="ExternalInput")
    # Wqk packed in consumption-ordered chunks: chunk c = (colhalf a, dcpair
    # dp) at c = a*4+dp, holding [{row0,row1}, 512 cols] (hi only; the G
    # GEMM runs single-fp8 -- Xq and Wqk each quantized once).
    wq_in = nc.dram_tensor("wq", [P, 8, 2, 512], F8, kind="ExternalInput")
    wvh_in = nc.dram_tensor("wvh", [P, DC, D], F8, kind="ExternalInput")
    wvl_in = nc.dram_tensor("wvl", [P, DC, D], F8, kind="ExternalInput")
    mask_a_in = nc.dram_tensor("mask_a", [P, P], BF16, kind="ExternalInput")
    mask_b_in = nc.dram_tensor("mask_b", [P, P], BF16, kind="ExternalInput")
    ones_in = nc.dram_tensor("ones", [P, 1], BF16, kind="ExternalInput")
    scl_in = nc.dram_tensor("scl", [P, 4], F32, kind="ExternalInput")
    out = nc.dram_tensor("out", [NQ, D], BF16, kind="ExternalOutput")

    with tile.TileContext(nc) as tc:
        with (
            tc.tile_pool(name="res", bufs=1) as res,
            tc.tile_pool(name="psum", bufs=4, space="PSUM") as pp,
        ):
            kt_sb = res.tile([P, EC, S], F8)        # K^T hi [e, keys]
            ktl_sb = res.tile([P, EC, S], F8)       # K^T lo (slots 4-7)
            xv_sb = res.tile([P, S // P, 2, D], F8)  # X_v hi/lo interleaved
            xq_sb = res.tile([P, DC, NQ], F8)       # X_q^T [d, q]
            qth_sb = res.tile([P, EC, NQ], F8)      # Q^T hi [e, q]
            qtl_sb = res.tile([P, EC, 512], F8)     # Q^T lo (qb0 only)
            wq_sb = res.tile([P, 8, 2, 512], F8)  # Wqk packed chunks
            wvh_sb = res.tile([P, DC, D], F8)
            wvl_sb = res.tile([P, DC, D], F8)
            uth_sb = res.tile([P, NSLOT, DC, P], F8)  # U^T hi (all slots)
            utl_sb = res.tile([P, NSLOT, DC, P], F8)
            rec_sb = res.tile([P, NSLOT], F32)        # 1/(c*den) per slot
            mask_a = res.tile([P, P], BF16)
            mask_b = res.tile([P, P], BF16)
            ones_sb = res.tile([P, 1], BF16)
            scl_sb = res.tile([P, 4], F32)
            s_exp = scl_sb[:, 0:1]   # exp scale = SCALE/(ag*ak)
            s_qt = scl_sb[:, 1:2]    # gamma_g: psum->qt scale
            s_ut = scl_sb[:, 2:3]    # gamma_u: psum->ut scale

            # ---- DMA: Xq-qb0 + consts on scalar; all bulk on sync, in
            # first-consumption order.  Each issue costs ~0.63us on its
            # queue, so the stream uses few, large chunks. ----
            nc.scalar.dma_start(xq_sb[:, 0:2, 0:512], xq_in[:, 0:2, 0:512])
            nc.gpsimd.dma_start(xq_sb[:, 2:8, 0:512], xq_in[:, 2:8, 0:512])
            nc.gpsimd.dma_start(scl_sb, scl_in[:, :])
            nc.gpsimd.dma_start(ones_sb, ones_in[:, :])
            nc.gpsimd.dma_start(mask_a, mask_a_in[:, :])
            nc.gpsimd.dma_start(mask_b, mask_b_in[:, :])
            nc.sync.dma_start(wq_sb[:, 0:1], wq_in[:, 0:1])
            nc.sync.dma_start(wq_sb[:, 1:2], wq_in[:, 1:2])
            nc.sync.dma_start(wq_sb[:, 2:3], wq_in[:, 2:3])
            nc.sync.dma_start(wq_sb[:, 3:4], wq_in[:, 3:4])
            nc.sync.dma_start(wq_sb[:, 4:5], wq_in[:, 4:5])
            nc.sync.dma_start(wq_sb[:, 5:6], wq_in[:, 5:6])
            nc.sync.dma_start(wq_sb[:, 6:7], wq_in[:, 6:7])
            nc.sync.dma_start(wq_sb[:, 7:8], wq_in[:, 7:8])
            nc.sync.dma_start(kt_sb[:, :, 0:512], xk_in[:, :, 0:512])
            nc.sync.dma_start(xv_sb[:, 0:2], xv_in[:, 0:2])
            nc.sync.dma_start(kt_sb[:, :, 512:1024], xk_in[:, :, 512:1024])
            nc.sync.dma_start(xv_sb[:, 2:4], xv_in[:, 2:4])
            nc.sync.dma_start(xq_sb[:, :, 512:NQ], xq_in[:, :, 512:NQ])
            nc.sync.dma_start(kt_sb[:, :, 1024:1536], xk_in[:, :, 1024:1536])
            nc.sync.dma_start(xv_sb[:, 4:8], xv_in[:, 4:8])
            nc.sync.dma_start(ktl_sb[:, :, 0:1024], xkl_in[:, :, 0:1024])
            nc.sync.dma_start(kt_sb[:, :, 1536:2048], xk_in[:, :, 1536:2048])
            nc.sync.dma_start(xv_sb[:, 8:12], xv_in[:, 8:12])
            nc.sync.dma_start(ktl_sb[:, :, 1024:2048], xkl_in[:, :, 1024:2048])
            nc.sync.dma_start(xv_sb[:, 12:16], xv_in[:, 12:16])
            nc.sync.dma_start(wvh_sb[:, :], wvh_in[:, :])
            nc.sync.dma_start(wvl_sb[:, :], wvl_in[:, :])

            def qt_copy(ec, cols, ps, lo=True, dve_hi=False):
                if dve_hi:
                    nc.vector.tensor_scalar_mul(
                        qth_sb[:, ec, cols], ps, s_qt
                    )
                else:
                    nc.scalar.activation(
                        out=qth_sb[:, ec, cols], in_=ps,
                        func=mybir.ActivationFunctionType.Copy, scale=s_qt,
                    )
                if lo:
                    nc.vector.scalar_tensor_tensor(
                        out=qtl_sb[:, ec, cols], in0=ps, scalar=s_qt,
                        in1=qth_sb[:, ec, cols], op0=MULT, op1=SUB,
                    )

            # ============ G^T phase ============
            # qb=0 (q cols 0:512 = slots 0-3, consumed first) dc-pair-outer
            # so compute starts as soon as the first wqk/xq chunks land.
            # wq_sb chunk c = (colhalf a)*4 + dp holds [{hi,lo}, pair, 512].
            for a in range(2):  # column half == ec group
                ps_list = [
                    pp.tile([P, 512], F32, tag="ps", bufs=5, name=f"psg{a}_{i}")
                    for i in range(4)
                ]
                for dp in range(DCP):
                    for i in range(4):
                        co = slice(i * P, (i + 1) * P)
                        nc.tensor.matmul(
                            ps_list[i], wq_sb[:, a * 4 + dp, :, co],
                            xq_sb[:, 2 * dp:2 * dp + 2, 0:512],
                            start=(dp == 0), stop=(dp == DCP - 1),
                            perf_mode=DR,
                        )
                for i in range(4):
                    qt_copy(a * 4 + i, slice(0, 512), ps_list[i])
            # qb=1 (q cols 512:1024) ec-outer; emitted lazily after slot 3
            # so the early slots aren't gated on the qb=1 Xq DMA
            def emit_gqb1(ec0, ec1):
                for ec in range(ec0, ec1):
                    ps = pp.tile([P, 512], F32, tag="ps", bufs=5)
                    a, i = ec // 4, ec % 4
                    co = slice(i * P, (i + 1) * P)
                    for dp in range(DCP):
                        nc.tensor.matmul(
                            ps, wq_sb[:, a * 4 + dp, :, co],
                            xq_sb[:, 2 * dp:2 * dp + 2, 512:NQ],
                            start=(dp == 0), stop=(dp == DCP - 1),
                            perf_mode=DR,
                        )
                    qt_copy(ec, slice(512, NQ), ps, lo=False,
                            dve_hi=(ec % 2 == 1))

            # ================= attention =================
            with tc.tile_pool(name="attn", bufs=2) as ap:
                slot_state = {}
                pending = []

                def consume(s, g):
                    st = slot_state[s]
                    nt = st["nt"]
                    cnt = min(4, nt - g * 4)
                    a16, ah, al = st["a16"], st["ah"], st["al"]
                    for i in range(cnt):
                        t = g * 4 + i
                        nc.tensor.matmul(
                            st["ps_den"], a16[:, t, :], ones_sb,
                            start=(t == 0), stop=(t == nt - 1),
                        )
                    if g != st["ng"] - 1:
                        return
                    # slot finished: U^T hi/lo; Y deferred to the end
                    nc.vector.reciprocal(
                        rec_sb[:, s:s + 1], st["ps_den"]
                    )
                    np_ = nt // 2
                    for dq in range(2):
                        ps_u = pp.tile(
                            [P, 512], F32, tag="ua", bufs=2,
                            name=f"psu{s}_{dq}",
                        )
                        for i in range(4):
                            dc = dq * 4 + i
                            ds = slice(dc * P, (dc + 1) * P)
                            po = slice(i * P, (i + 1) * P)
                            for kp in range(np_):
                                ks = slice(2 * kp, 2 * kp + 2)
                                nc.tensor.matmul(
                                    ps_u[:, po], xv_sb[:, ks, 0, ds],
                                    ah[:, ks, :],
                                    start=(kp == 0), stop=False, perf_mode=DR,
                                )
                                nc.tensor.matmul(
                                    ps_u[:, po], xv_sb[:, ks, 0, ds],
                                    al[:, ks, :],
                                    start=False, stop=False, perf_mode=DR,
                                )
                                nc.tensor.matmul(
                                    ps_u[:, po], xv_sb[:, ks, 1, ds],
                                    ah[:, ks, :],
                                    start=False, stop=(kp == np_ - 1),
                                    perf_mode=DR,
                                )
                        ucols = slice(dq * 4, dq * 4 + 4)
                        nc.scalar.activation(
                            out=uth_sb[:, s, ucols, :], in_=ps_u,
                            func=mybir.ActivationFunctionType.Copy,
                            scale=s_ut,
                        )
                        nc.vector.scalar_tensor_tensor(
                            out=utl_sb[:, s, ucols, :], in0=ps_u,
                            scalar=s_ut, in1=uth_sb[:, s, ucols, :],
                            op0=MULT, op1=SUB,
                        )
                    del slot_state[s]

                for s in range(NSLOT):
                    if s == 4:
                        emit_gqb1(0, 8)
                        while pending:
                            consume(*pending.pop(0))
                    L = 256 * (s + 1)
                    nt = L // P
                    ng = (nt + 3) // 4
                    slot_state[s] = {
                        "nt": nt,
                        "ng": ng,
                        "a16": ap.tile(
                            [P, S // P, P], BF16, tag="a16", bufs=3,
                            name=f"a16_{s}",
                        ),
                        "ah": ap.tile(
                            [P, S // P, P], F8, tag="ah", bufs=3,
                            name=f"ah_{s}",
                        ),
                        "al": ap.tile(
                            [P, S // P, P], F8, tag="al", bufs=3,
                            name=f"al_{s}",
                        ),
                        "ps_den": pp.tile(
                            [P, 1], F32, tag="psden", bufs=1, name=f"psden{s}"
                        ),
                    }
                    st = slot_state[s]
                    qs = slice(s * P, (s + 1) * P)
                    for g in range(ng):
                        cnt = min(4, nt - g * 4)
                        psT = pp.tile([P, 512], F32, tag="ps", bufs=5)
                        for i in range(cnt):
                            t = g * 4 + i
                            po = slice(i * P, (i + 1) * P)
                            ts = slice(t * P, (t + 1) * P)
                            for j in range(DCP):
                                sl = slice(2 * j, 2 * j + 2)
                                nc.tensor.matmul(
                                    psT[:, po], kt_sb[:, sl, ts],
                                    qth_sb[:, sl, qs],
                                    start=(j == 0), stop=False, perf_mode=DR,
                                )
                                if s < 4:
                                    nc.tensor.matmul(
                                        psT[:, po], kt_sb[:, sl, ts],
                                        qtl_sb[:, sl, qs],
                                        start=False, stop=(j == DCP - 1),
                                        perf_mode=DR,
                                    )
                                else:
                                    nc.tensor.matmul(
                                        psT[:, po], ktl_sb[:, sl, ts],
                                        qth_sb[:, sl, qs],
                                        start=False, stop=(j == DCP - 1),
                                        perf_mode=DR,
                                    )
                        if g == ng - 1:
                            nc.vector.tensor_add(
                                out=psT[:, (cnt - 2) * P:(cnt - 1) * P],
                                in0=psT[:, (cnt - 2) * P:(cnt - 1) * P],
                                in1=mask_a,
                            )
                            nc.vector.tensor_add(
                                out=psT[:, (cnt - 1) * P:cnt * P],
                                in0=psT[:, (cnt - 1) * P:cnt * P],
                                in1=mask_b,
                            )
                        gs = slice(g * 4, g * 4 + cnt)
                        nc.scalar.activation(
                            out=st["a16"][:, gs, :], in_=psT[:, :cnt * P],
                            func=mybir.ActivationFunctionType.Exp,
                            scale=s_exp,
                        )
                        nc.scalar.activation(
                            out=st["ah"][:, gs, :], in_=psT[:, :cnt * P],
                            func=mybir.ActivationFunctionType.Exp,
                            scale=s_exp,
                        )
                        nc.vector.tensor_sub(
                            out=st["al"][:, gs, :],
                            in0=st["a16"][:, gs, :],
                            in1=st["ah"][:, gs, :],
                        )
                        if len(pending) >= 2:
                            consume(*pending.pop(0))
                        pending.append((s, g))
                while pending:
                    consume(*pending.pop(0))

                # ---- deferred Y = U @ Wv for all slots ----
                for s in range(NSLOT):
                    out_sb = ap.tile([P, D], BF16, tag="out", bufs=4)
                    rec = rec_sb[:, s:s + 1]
                    for eq in range(4):  # 256-wide chains: copy+DMA start
                        es = slice(eq * 256, (eq + 1) * 256)  # mid-GEMM
                        ps_y = pp.tile(
                            [P, 256], F32, tag="ps", bufs=5,
                            name=f"psy{eq}_{s}",
                        )
                        for dp in range(DCP):
                            sl = slice(2 * dp, 2 * dp + 2)
                            nc.tensor.matmul(
                                ps_y, uth_sb[:, s, sl, :], wvh_sb[:, sl, es],
                                start=(dp == 0), stop=False, perf_mode=DR,
                            )
                            nc.tensor.matmul(
                                ps_y, uth_sb[:, s, sl, :], wvl_sb[:, sl, es],
                                start=False, stop=False, perf_mode=DR,
                            )
                            nc.tensor.matmul(
                                ps_y, utl_sb[:, s, sl, :], wvh_sb[:, sl, es],
                                start=False, stop=(dp == DCP - 1),
                                perf_mode=DR,
                            )
                        if eq % 2 == 0:
                            nc.scalar.activation(
                                out=out_sb[:, es], in_=ps_y,
                                func=mybir.ActivationFunctionType.Copy,
                                scale=rec,
                            )
                        else:
                            nc.vector.tensor_scalar_mul(
                                out_sb[:, es], ps_y, rec
                            )
                        eng = nc.gpsimd if eq % 2 == 0 else nc.sync
                        eng.dma_start(
                            out[s * P:(s + 1) * P, es], out_sb[:, es]
                        )

    nc.compile()
    return nc


_NC_CACHE = None


def _get_nc():
    global _NC_CACHE
    if _NC_CACHE is None:
        _NC_CACHE = build_attention_nc()
    return _NC_CACHE


def _make_masks(h):
    """Transposed masks [key kk, query r] for the last two key tiles."""
    import ml_dtypes

    kk = np.arange(P)[:, None]
    r = np.arange(P)[None, :]
    tri = np.where(kk <= r, 0.0, NEG).astype(np.float32)
    if h == 0:
        mask_a, mask_b = tri, np.full((P, P), NEG, dtype=np.float32)
    else:
        mask_a, mask_b = np.zeros((P, P), dtype=np.float32), tri
    return mask_a.astype(ml_dtypes.bfloat16), mask_b.astype(ml_dtypes.bfloat16)


def _pow2_floor(x):
    return float(2.0 ** np.floor(np.log2(x)))


def kernel(
    inputs_for_keys,
    inputs_for_values,
    inputs_for_queries,
    weight_K,
    weight_V,
    weight_Q,
    trace=False,
):
    import ml_dtypes

    f8 = ml_dtypes.float8_e4m3

    xk_full = np.asarray(inputs_for_keys, dtype=np.float32)
    xv_full = np.asarray(inputs_for_values, dtype=np.float32)
    xq_full = np.asarray(inputs_for_queries, dtype=np.float32)
    w_v = np.asarray(weight_V, dtype=np.float32)
    w_qk = (
        np.asarray(weight_Q, dtype=np.float32)
        @ np.asarray(weight_K, dtype=np.float32).T
    )

    # power-of-two scales (range only; fp8 rel precision is scale-free)
    aq = _pow2_floor(192.0 / max(np.abs(xq_full).max(), 1e-30))
    ak = _pow2_floor(192.0 / max(np.abs(xk_full).max(), 1e-30))
    av = _pow2_floor(192.0 / max(np.abs(xv_full).max(), 1e-30))
    aqk = _pow2_floor(192.0 / max(np.abs(w_qk).max(), 1e-30))
    aw = _pow2_floor(192.0 / max(np.abs(w_v).max(), 1e-30))
    # G row scale: G = Xq@Wqk is near-Gaussian; absmax ~<= 8*sigma
    sg = float(
        np.sqrt((w_qk ** 2).mean() * D * (xq_full ** 2).mean())
    )
    ag = _pow2_floor(192.0 / (8.0 * sg))
    # U = attn@Xv (unnormalized): sigma_U^2 ~= sum_k E[a^2] * sigma_v^2
    ss = SCALE * sg * float(np.sqrt((xk_full ** 2).mean() * D))
    ea2 = float(np.exp(2.0 * ss * ss))
    su = float(np.sqrt(S * ea2 * (xv_full ** 2).mean()))
    au = _pow2_floor(16.0 / su)

    s_exp = SCALE / (ag * ak)
    g_qt = ag / (aq * aqk)      # psum(G*aq*aqk) -> qt = G*ag
    g_ut = au / av              # psum(U*av) -> ut = U*au
    c_ones = au * aw            # den vector value; rec = 1/(c*den)

    def hl(x, scale, layout):
        xs = np.asarray(x, np.float32) * scale
        hi = xs.astype(f8)
        lo = (xs - hi.astype(np.float32)).astype(f8)
        return layout(hi), layout(lo)

    def _xT(x):  # [rows, D] -> [P, DC, rows] (d on partitions)
        xt = np.asarray(x).T.reshape(DC, P, x.shape[0])
        return np.ascontiguousarray(xt.transpose(1, 0, 2))

    def _w(w):  # [D, D] -> [P, DC, D]
        wr = np.asarray(w).reshape(DC, P, D)
        return np.ascontiguousarray(wr.transpose(1, 0, 2))

    def _xv(x):  # [S, D] -> [P, S//P, D] (keys on partitions)
        xr = np.asarray(x).reshape(S // P, P, D)
        return np.ascontiguousarray(xr.transpose(1, 0, 2))

    wqh = _w((w_qk * aqk).astype(f8))
    wvh, wvl = hl(w_v, aw, _w)
    # pack wqk chunks [P, 8, 2, 512]: chunk (a*4+dp) = row-pair x col half a
    wq_pack = np.empty((P, 8, 2, 512), dtype=f8)
    for a in range(2):
        for dp in range(DCP):
            cs = slice(a * 512, (a + 1) * 512)
            wq_pack[:, a * 4 + dp] = wqh[:, 2 * dp:2 * dp + 2, cs]
    xkp = [hl(xk_full[b], ak, _xT) for b in range(B)]
    xvp = []
    for b in range(B):
        h_, l_ = hl(xv_full[b], av, _xv)
        xi = np.empty((P, S // P, 2, D), dtype=f8)
        xi[:, :, 0] = h_
        xi[:, :, 1] = l_
        xvp.append(xi)

    masks = [_make_masks(0), _make_masks(1)]
    ones_np = np.full((P, 1), c_ones, np.float32).astype(ml_dtypes.bfloat16)
    scl_np = np.zeros((P, 4), np.float32)
    scl_np[:, 0] = s_exp
    scl_np[:, 1] = g_qt
    scl_np[:, 2] = g_ut

    in_maps = []
    for c in range(2 * B):
        b, h = c // 2, c % 2
        rows = np.concatenate(
            [
                xq_full[b, 256 * s + 128 * h: 256 * s + 128 * h + P, :]
                for s in range(NSLOT)
            ],
            axis=0,
        )
        xqc = _xT((rows * aq).astype(f8))
        in_maps.append(
            {
                "xq": xqc,
                "xk": xkp[b][0],
                "xkl": xkp[b][1],
                "xv": xvp[b],
                "wq": wq_pack,
                "wvh": wvh,
                "wvl": wvl,
                "mask_a": masks[h][0],
                "mask_b": masks[h][1],
                "ones": ones_np,
                "scl": scl_np,
            }
        )

    nc = _get_nc()
    res = bass_utils.run_bass_kernel_spmd(
        nc, in_maps, core_ids=list(range(2 * B)), trace=trace
    )

    out = np.empty((B, S, D), dtype=np.float32)
    for c in range(2 * B):
        b, h = c // 2, c % 2
        o = np.asarray(res.results[c]["out"], dtype=np.float32)
        for s in range(NSLOT):
            out[b, 256 * s + 128 * h: 256 * s + 128 * h + P, :] = o[
                s * P:(s + 1) * P, :
            ]

    if trace:
        return out, res
    return out


# revision 11
# speedup vs baseline: 1.0276x; 1.0037x over previous
"""Causal single-head attention (B=4, S=2048, D=1024) on 8 TRN2 NeuronCores.

fp8(e4m3) DoubleRow rewrite of the bf16 baseline.  Same sharding: core
c -> (batch b = c//2, half h = c%2); 8 query slots of 128 rows with padded
causal key-lengths L_s = 256*(s+1); scores computed transposed; Wqk = Wq@Wk^T
fused host-side; out = ((attn @ X_v) @ W_v) / den.

Quantization scheme (all matmuls fp8e4 DoubleRow, PSUM fp32):
  G = Xq @ Wqk      single-fp8 (Wqk hi only, Xq fp8); G -> qt hi+lo on
                    device (ACT copy + DVE scalar_tensor_tensor residual).
  scores = qt@K^T   2-product: qt hi+lo (device), K^T single fp8 (host).
  U = attn @ Xv     3-product: attn hi+lo (ACT exp f8 + bf16, DVE sub),
                    Xv hi+lo (host).  U -> ut hi+lo on device.
  Y = U @ Wv        3-product: ut hi+lo (device), Wv hi+lo (host).
hi+lo fp8 pairs represent bf16 values exactly, so the residual GEMMs have
~bf16-level error; the 2-product GEMMs err only by the single side's fp8
quantization.  The softmax denominator is summed from the quantized bf16
attn (== hi+lo exactly), cancelling common-mode quantization error; the
"ones" den vector carries value au*aw so 1/den absorbs all output scales.
Scales are powers of two shipped in a [P,4] f32 vector, so the compiled
program is input-independent.

Schedule: the cost model serializes all DMA on one shared resource
(~360 GB/s) with ~0.6us per-issue overhead, so all bulk input DMA goes on
ONE queue (sync) in exact first-consumption order; consts + Xq ride the
scalar queue; output DMA on the SWDGE.  Slots run ASCENDING (0..7) so K/V
chunks stream just-in-time, and the 8 Y GEMMs are deferred to the end
(ut hi/lo tiles are tiny and stay resident) where they keep the PE busy
through the tail while Wv has long arrived.

NOTE (hazard, empirical): interleaving start/stop matmul accumulation chains
across sub-regions of a single PSUM bank corrupts results on HW; keep each
sub-tile's chain contiguous (interleaving across banks is fine).
"""

import numpy as np

import concourse.bacc as bacc
import concourse.mybir as mybir
import concourse.tile as tile
from concourse import bass_utils

B, S, D = 4, 2048, 1024
P = 128
DC = D // P          # 8 contraction chunks
DCP = DC // 2        # 4 DoubleRow contraction pairs
EC = D // P
NSLOT = 8
NQ = NSLOT * P
SCALE = 1.0 / float(np.sqrt(np.float32(S)))
NEG = -1.0e9

F32 = mybir.dt.float32
BF16 = mybir.dt.bfloat16
F8 = mybir.dt.float8e4
DR = mybir.MatmulPerfMode.DoubleRow
MULT = mybir.AluOpType.mult
SUB = mybir.AluOpType.subtract


def build_attention_nc():
    nc = bacc.Bacc("TRN2", target_bir_lowering=False)

    xq_in = nc.dram_tensor("xq", [P, DC, NQ], F8, kind="ExternalInput")
    xk_in = nc.dram_tensor("xk", [P, EC, S], F8, kind="ExternalInput")
    xkl_in = nc.dram_tensor("xkl", [P, EC, S], F8, kind="ExternalInput")
    # X_v hi/lo interleaved per key tile: [p, tile, {hi,lo}, d]
    xv_in = nc.dram_tensor("xv", [P, S // P, 2, D], F8, kind="ExternalInput")
    # Wqk packed in consumption-ordered chunks: chunk c = (colhalf a, dcpair
    # dp) at c = a*4+dp, holding [{row0,row1}, 512 cols] (hi only; the G
    # GEMM runs single-fp8 -- Xq and Wqk each quantized once).
    wq_in = nc.dram_tensor("wq", [P, 8, 2, 512], F8, kind="ExternalInput")
    wvh_in = nc.dram_tensor("wvh", [P, DC, D], F8, kind="ExternalInput")
    wvl_in = nc.dram_tensor("wvl", [P, DC, D], F8, kind="ExternalInput")
    mask_a_in = nc.dram_tensor("mask_a", [P, P], BF16, kind="ExternalInput")
    mask_b_in = nc.dram_tensor("mask_b", [P, P], BF16, kind="ExternalInput")
    ones_in = nc.dram_tensor("ones", [P, 1], BF16, kind="ExternalInput")
    scl_in = nc.dram_tensor("scl", [P, 4], F32, kind="ExternalInput")
    out = nc.dram_tensor("out", [NQ, D], BF16, kind="ExternalOutput")

    with tile.TileContext(nc) as tc:
        with (
            tc.tile_pool(name="res", bufs=1) as res,
            tc.tile_pool(name="psum", bufs=4, space="PSUM") as pp,
        ):
            kt_sb = res.tile([P, EC, S], F8)        # K^T hi [e, keys]
            ktl_sb = res.tile([P, EC, S], F8)       # K^T lo (slots 4-7)
            xv_sb = res.tile([P, S // P, 2, D], F8)  # X_v hi/lo interleaved
            xq_sb = res.tile([P, DC, NQ], F8)       # X_q^T [d, q]
            qth_sb = res.tile([P, EC, NQ], F8)      # Q^T hi [e, q]
            qtl_sb = res.tile([P, EC, 512], F8)     # Q^T lo (qb0 only)
            wq_sb = res.tile([P, 8, 2, 512], F8)  # Wqk packed chunks
            wvh_sb = res.tile([P, DC, D], F8)
            wvl_sb = res.tile([P, DC, D], F8)
            uth_sb = res.tile([P, NSLOT, DC, P], F8)  # U^T hi (all slots)
            utl_sb = res.tile([P, NSLOT, DC, P], F8)
            rec_sb = res.tile([P, NSLOT], F32)        # 1/(c*den) per slot
            mask_a = res.tile([P, P], BF16)
            mask_b = res.tile([P, P], BF16)
            ones_sb = res.tile([P, 1], BF16)
            scl_sb = res.tile([P, 4], F32)
            s_exp = scl_sb[:, 0:1]   # exp scale = SCALE/(ag*ak)
            s_qt = scl_sb[:, 1:2]    # gamma_g: psum->qt scale
            s_ut = scl_sb[:, 2:3]    # gamma_u: psum->ut scale

            # ---- DMA: Xq-qb0 + consts on scalar; all bulk on sync, in
            # first-consumption order.  Each issue costs ~0.63us on its
            # queue, so the stream uses few, large chunks. ----
            nc.scalar.dma_start(xq_sb[:, 0:2, 0:512], xq_in[:, 0:2, 0:512])
            nc.gpsimd.dma_start(xq_sb[:, 2:8, 0:512], xq_in[:, 2:8, 0:512])
            nc.gpsimd.dma_start(scl_sb, scl_in[:, :])
            nc.gpsimd.dma_start(ones_sb, ones_in[:, :])
            nc.gpsimd.dma_start(mask_a, mask_a_in[:, :])
            nc.gpsimd.dma_start(mask_b, mask_b_in[:, :])
            nc.sync.dma_start(wq_sb[:, 0:1], wq_in[:, 0:1])
            nc.sync.dma_start(wq_sb[:, 1:2], wq_in[:, 1:2])
            nc.sync.dma_start(wq_sb[:, 2:3], wq_in[:, 2:3])
            nc.sync.dma_start(wq_sb[:, 3:4], wq_in[:, 3:4])
            nc.sync.dma_start(wq_sb[:, 4:5], wq_in[:, 4:5])
            nc.sync.dma_start(wq_sb[:, 5:6], wq_in[:, 5:6])
            nc.sync.dma_start(wq_sb[:, 6:7], wq_in[:, 6:7])
            nc.sync.dma_start(wq_sb[:, 7:8], wq_in[:, 7:8])
            nc.sync.dma_start(kt_sb[:, :, 0:512], xk_in[:, :, 0:512])
            nc.sync.dma_start(kt_sb[:, :, 512:1024], xk_in[:, :, 512:1024])
            nc.sync.dma_start(xv_sb[:, 0:2], xv_in[:, 0:2])
            nc.sync.dma_start(xv_sb[:, 2:4], xv_in[:, 2:4])
            nc.sync.dma_start(xq_sb[:, :, 512:NQ], xq_in[:, :, 512:NQ])
            nc.sync.dma_start(kt_sb[:, :, 1024:1536], xk_in[:, :, 1024:1536])
            nc.sync.dma_start(xv_sb[:, 4:8], xv_in[:, 4:8])
            nc.sync.dma_start(ktl_sb[:, :, 0:1024], xkl_in[:, :, 0:1024])
            nc.sync.dma_start(kt_sb[:, :, 1536:2048], xk_in[:, :, 1536:2048])
            nc.sync.dma_start(xv_sb[:, 8:12], xv_in[:, 8:12])
            nc.sync.dma_start(ktl_sb[:, :, 1024:2048], xkl_in[:, :, 1024:2048])
            nc.sync.dma_start(xv_sb[:, 12:16], xv_in[:, 12:16])
            nc.sync.dma_start(wvh_sb[:, :], wvh_in[:, :])
            nc.sync.dma_start(wvl_sb[:, :], wvl_in[:, :])

            def qt_copy(ec, cols, ps, lo=True, dve_hi=False):
                if dve_hi:
                    nc.vector.tensor_scalar_mul(
                        qth_sb[:, ec, cols], ps, s_qt
                    )
                else:
                    nc.scalar.activation(
                        out=qth_sb[:, ec, cols], in_=ps,
                        func=mybir.ActivationFunctionType.Copy, scale=s_qt,
                    )
                if lo:
                    nc.vector.scalar_tensor_tensor(
                        out=qtl_sb[:, ec, cols], in0=ps, scalar=s_qt,
                        in1=qth_sb[:, ec, cols], op0=MULT, op1=SUB,
                    )

            # ============ G^T phase ============
            # qb=0 (q cols 0:512 = slots 0-3, consumed first) dc-pair-outer
            # so compute starts as soon as the first wqk/xq chunks land.
            # wq_sb chunk c = (colhalf a)*4 + dp holds [{hi,lo}, pair, 512].
            for a in range(2):  # column half == ec group
                ps_list = [
                    pp.tile([P, 512], F32, tag="ps", bufs=5, name=f"psg{a}_{i}")
                    for i in range(4)
                ]
                for dp in range(DCP):
                    for i in range(4):
                        co = slice(i * P, (i + 1) * P)
                        nc.tensor.matmul(
                            ps_list[i], wq_sb[:, a * 4 + dp, :, co],
                            xq_sb[:, 2 * dp:2 * dp + 2, 0:512],
                            start=(dp == 0), stop=(dp == DCP - 1),
                            perf_mode=DR,
                        )
                for i in range(4):
                    qt_copy(a * 4 + i, slice(0, 512), ps_list[i])
            # qb=1 (q cols 512:1024) ec-outer; emitted lazily after slot 3
            # so the early slots aren't gated on the qb=1 Xq DMA
            def emit_gqb1(ec0, ec1):
                for ec in range(ec0, ec1):
                    ps = pp.tile([P, 512], F32, tag="ps", bufs=5)
                    a, i = ec // 4, ec % 4
                    co = slice(i * P, (i + 1) * P)
                    for dp in range(DCP):
                        nc.tensor.matmul(
                            ps, wq_sb[:, a * 4 + dp, :, co],
                            xq_sb[:, 2 * dp:2 * dp + 2, 512:NQ],
                            start=(dp == 0), stop=(dp == DCP - 1),
                            perf_mode=DR,
                        )
                    qt_copy(ec, slice(512, NQ), ps, lo=False,
                            dve_hi=(ec % 2 == 1))

            # ================= attention =================
            with tc.tile_pool(name="attn", bufs=2) as ap:
                slot_state = {}
                pending = []

                def consume(s, g):
                    st = slot_state[s]
                    nt = st["nt"]
                    cnt = min(4, nt - g * 4)
                    a16, ah, al = st["a16"], st["ah"], st["al"]
                    for i in range(cnt):
                        t = g * 4 + i
                        nc.tensor.matmul(
                            st["ps_den"], a16[:, t, :], ones_sb,
                            start=(t == 0), stop=(t == nt - 1),
                        )
                    if g != st["ng"] - 1:
                        return
                    # slot finished: U^T hi/lo; Y deferred to the end
                    nc.vector.reciprocal(
                        rec_sb[:, s:s + 1], st["ps_den"]
                    )
                    np_ = nt // 2
                    for dq in range(2):
                        ps_u = pp.tile(
                            [P, 512], F32, tag="ua", bufs=2,
                            name=f"psu{s}_{dq}",
                        )
                        for i in range(4):
                            dc = dq * 4 + i
                            ds = slice(dc * P, (dc + 1) * P)
                            po = slice(i * P, (i + 1) * P)
                            for kp in range(np_):
                                ks = slice(2 * kp, 2 * kp + 2)
                                nc.tensor.matmul(
                                    ps_u[:, po], xv_sb[:, ks, 0, ds],
                                    ah[:, ks, :],
                                    start=(kp == 0), stop=False, perf_mode=DR,
                                )
                                nc.tensor.matmul(
                                    ps_u[:, po], xv_sb[:, ks, 0, ds],
                                    al[:, ks, :],
                                    start=False, stop=False, perf_mode=DR,
                                )
                                nc.tensor.matmul(
                                    ps_u[:, po], xv_sb[:, ks, 1, ds],
                                    ah[:, ks, :],
                                    start=False, stop=(kp == np_ - 1),
                                    perf_mode=DR,
                                )
                        ucols = slice(dq * 4, dq * 4 + 4)
                        nc.scalar.activation(
                            out=uth_sb[:, s, ucols, :], in_=ps_u,
                            func=mybir.ActivationFunctionType.Copy,
                            scale=s_ut,
                        )
                        nc.vector.scalar_tensor_tensor(
                            out=utl_sb[:, s, ucols, :], in0=ps_u,
                            scalar=s_ut, in1=uth_sb[:, s, ucols, :],
                            op0=MULT, op1=SUB,
                        )
                    del slot_state[s]

                for s in range(NSLOT):
                    if s == 4:
                        emit_gqb1(0, 8)
                        while pending:
                            consume(*pending.pop(0))
                    L = 256 * (s + 1)
                    nt = L // P
                    ng = (nt + 3) // 4
                    slot_state[s] = {
                        "nt": nt,
                        "ng": ng,
                        "a16": ap.tile(
                            [P, S // P, P], BF16, tag="a16", bufs=3,
                            name=f"a16_{s}",
                        ),
                        "ah": ap.tile(
                            [P, S // P, P], F8, tag="ah", bufs=3,
                            name=f"ah_{s}",
                        ),
                        "al": ap.tile(
                            [P, S // P, P], F8, tag="al", bufs=3,
                            name=f"al_{s}",
                        ),
                        "ps_den": pp.tile(
                            [P, 1], F32, tag="psden", bufs=1, name=f"psden{s}"
                        ),
                    }
                    st = slot_state[s]
                    qs = slice(s * P, (s + 1) * P)
                    for g in range(ng):
                        cnt = min(4, nt - g * 4)
                        psT = pp.tile([P, 512], F32, tag="ps", bufs=5)
                        for i in range(cnt):
                            t = g * 4 + i
                            po = slice(i * P, (i + 1) * P)
                            ts = slice(t * P, (t + 1) * P)
                            for j in range(DCP):
                                sl = slice(2 * j, 2 * j + 2)
                                nc.tensor.matmul(
                                    psT[:, po], kt_sb[:, sl, ts],
                                    qth_sb[:, sl, qs],
                                    start=(j == 0), stop=False, perf_mode=DR,
                                )
                                if s < 4:
                                    nc.tensor.matmul(
                                        psT[:, po], kt_sb[:, sl, ts],
                                        qtl_sb[:, sl, qs],
                                        start=False, stop=(j == DCP - 1),
                                        perf_mode=DR,
                                    )
                                else:
                                    nc.tensor.matmul(
                                        psT[:, po], ktl_sb[:, sl, ts],
                                        qth_sb[:, sl, qs],
                                        start=False, stop=(j == DCP - 1),
                                        perf_mode=DR,
                                    )
                        if g == ng - 1:
                            nc.vector.tensor_add(
                                out=psT[:, (cnt - 2) * P:(cnt - 1) * P],
                                in0=psT[:, (cnt - 2) * P:(cnt - 1) * P],
                                in1=mask_a,
                            )
                            nc.vector.tensor_add(
                                out=psT[:, (cnt - 1) * P:cnt * P],
                                in0=psT[:, (cnt - 1) * P:cnt * P],
                                in1=mask_b,
                            )
                        gs = slice(g * 4, g * 4 + cnt)
                        nc.scalar.activation(
                            out=st["a16"][:, gs, :], in_=psT[:, :cnt * P],
                            func=mybir.ActivationFunctionType.Exp,
                            scale=s_exp,
                        )
                        nc.scalar.activation(
                            out=st["ah"][:, gs, :], in_=psT[:, :cnt * P],
                            func=mybir.ActivationFunctionType.Exp,
                            scale=s_exp,
                        )
                        nc.vector.tensor_sub(
                            out=st["al"][:, gs, :],
                            in0=st["a16"][:, gs, :],
                            in1=st["ah"][:, gs, :],
                        )
                        if len(pending) >= 2:
                            consume(*pending.pop(0))
                        pending.append((s, g))
                while pending:
                    consume(*pending.pop(0))

                # ---- deferred Y = U @ Wv for all slots ----
                for s in range(NSLOT):
                    out_sb = ap.tile([P, D], BF16, tag="out", bufs=4)
                    rec = rec_sb[:, s:s + 1]
                    for eq in range(4):  # 256-wide chains: copy+DMA start
                        es = slice(eq * 256, (eq + 1) * 256)  # mid-GEMM
                        ps_y = pp.tile(
                            [P, 256], F32, tag="ps", bufs=5,
                            name=f"psy{eq}_{s}",
                        )
                        for dp in range(DCP):
                            sl = slice(2 * dp, 2 * dp + 2)
                            nc.tensor.matmul(
                                ps_y, uth_sb[:, s, sl, :], wvh_sb[:, sl, es],
                                start=(dp == 0), stop=False, perf_mode=DR,
                            )
                            nc.tensor.matmul(
                                ps_y, uth_sb[:, s, sl, :], wvl_sb[:, sl, es],
                                start=False, stop=False, perf_mode=DR,
                            )
                            nc.tensor.matmul(
                                ps_y, utl_sb[:, s, sl, :], wvh_sb[:, sl, es],
                                start=False, stop=(dp == DCP - 1),
                                perf_mode=DR,
                            )
                        if eq % 2 == 0:
                            nc.scalar.activation(
                                out=out_sb[:, es], in_=ps_y,
                                func=mybir.ActivationFunctionType.Copy,
                                scale=rec,
                            )
                        else:
                            nc.vector.tensor_scalar_mul(
                                out_sb[:, es], ps_y, rec
                            )
                        eng = nc.gpsimd if eq % 2 == 0 else nc.sync
                        eng.dma_start(
                            out[s * P:(s + 1) * P, es], out_sb[:, es]
                        )

    nc.compile()
    return nc


_NC_CACHE = None


def _get_nc():
    global _NC_CACHE
    if _NC_CACHE is None:
        _NC_CACHE = build_attention_nc()
    return _NC_CACHE


def _make_masks(h):
    """Transposed masks [key kk, query r] for the last two key tiles."""
    import ml_dtypes

    kk = np.arange(P)[:, None]
    r = np.arange(P)[None, :]
    tri = np.where(kk <= r, 0.0, NEG).astype(np.float32)
    if h == 0:
        mask_a, mask_b = tri, np.full((P, P), NEG, dtype=np.float32)
    else:
        mask_a, mask_b = np.zeros((P, P), dtype=np.float32), tri
    return mask_a.astype(ml_dtypes.bfloat16), mask_b.astype(ml_dtypes.bfloat16)


def _pow2_floor(x):
    return float(2.0 ** np.floor(np.log2(x)))


def kernel(
    inputs_for_keys,
    inputs_for_values,
    inputs_for_queries,
    weight_K,
    weight_V,
    weight_Q,
    trace=False,
):
    import ml_dtypes

    f8 = ml_dtypes.float8_e4m3

    xk_full = np.asarray(inputs_for_keys, dtype=np.float32)
    xv_full = np.asarray(inputs_for_values, dtype=np.float32)
    xq_full = np.asarray(inputs_for_queries, dtype=np.float32)
    w_v = np.asarray(weight_V, dtype=np.float32)
    w_qk = (
        np.asarray(weight_Q, dtype=np.float32)
        @ np.asarray(weight_K, dtype=np.float32).T
    )

    # power-of-two scales (range only; fp8 rel precision is scale-free)
    aq = _pow2_floor(192.0 / max(np.abs(xq_full).max(), 1e-30))
    ak = _pow2_floor(192.0 / max(np.abs(xk_full).max(), 1e-30))
    av = _pow2_floor(192.0 / max(np.abs(xv_full).max(), 1e-30))
    aqk = _pow2_floor(192.0 / max(np.abs(w_qk).max(), 1e-30))
    aw = _pow2_floor(192.0 / max(np.abs(w_v).max(), 1e-30))
    # G row scale: G = Xq@Wqk is near-Gaussian; absmax ~<= 8*sigma
    sg = float(
        np.sqrt((w_qk ** 2).mean() * D * (xq_full ** 2).mean())
    )
    ag = _pow2_floor(192.0 / (8.0 * sg))
    # U = attn@Xv (unnormalized): sigma_U^2 ~= sum_k E[a^2] * sigma_v^2
    ss = SCALE * sg * float(np.sqrt((xk_full ** 2).mean() * D))
    ea2 = float(np.exp(2.0 * ss * ss))
    su = float(np.sqrt(S * ea2 * (xv_full ** 2).mean()))
    au = _pow2_floor(16.0 / su)

    s_exp = SCALE / (ag * ak)
    g_qt = ag / (aq * aqk)      # psum(G*aq*aqk) -> qt = G*ag
    g_ut = au / av              # psum(U*av) -> ut = U*au
    c_ones = au * aw            # den vector value; rec = 1/(c*den)

    def hl(x, scale, layout):
        xs = np.asarray(x, np.float32) * scale
        hi = xs.astype(f8)
        lo = (xs - hi.astype(np.float32)).astype(f8)
        return layout(hi), layout(lo)

    def _xT(x):  # [rows, D] -> [P, DC, rows] (d on partitions)
        xt = np.asarray(x).T.reshape(DC, P, x.shape[0])
        return np.ascontiguousarray(xt.transpose(1, 0, 2))

    def _w(w):  # [D, D] -> [P, DC, D]
        wr = np.asarray(w).reshape(DC, P, D)
        return np.ascontiguousarray(wr.transpose(1, 0, 2))

    def _xv(x):  # [S, D] -> [P, S//P, D] (keys on partitions)
        xr = np.asarray(x).reshape(S // P, P, D)
        return np.ascontiguousarray(xr.transpose(1, 0, 2))

    wqh = _w((w_qk * aqk).astype(f8))
    wvh, wvl = hl(w_v, aw, _w)
    # pack wqk chunks [P, 8, 2, 512]: chunk (a*4+dp) = row-pair x col half a
    wq_pack = np.empty((P, 8, 2, 512), dtype=f8)
    for a in range(2):
        for dp in range(DCP):
            cs = slice(a * 512, (a + 1) * 512)
            wq_pack[:, a * 4 + dp] = wqh[:, 2 * dp:2 * dp + 2, cs]
    xkp = [hl(xk_full[b], ak, _xT) for b in range(B)]
    xvp = []
    for b in range(B):
        h_, l_ = hl(xv_full[b], av, _xv)
        xi = np.empty((P, S // P, 2, D), dtype=f8)
        xi[:, :, 0] = h_
        xi[:, :, 1] = l_
        xvp.append(xi)

    masks = [_make_masks(0), _make_masks(1)]
    ones_np = np.full((P, 1), c_ones, np.float32).astype(ml_dtypes.bfloat16)
    scl_np = np.zeros((P, 4), np.float32)
    scl_np[:, 0] = s_exp
    scl_np[:, 1] = g_qt
    scl_np[:, 2] = g_ut

    in_maps = []
    for c in range(2 * B):
        b, h = c // 2, c % 2
        rows = np.concatenate(
            [
                xq_full[b, 256 * s + 128 * h: 256 * s + 128 * h + P, :]
                for s in range(NSLOT)
            ],
            axis=0,
        )
        xqc = _xT((rows * aq).astype(f8))
        in_maps.append(
            {
                "xq": xqc,
                "xk": xkp[b][0],
                "xkl": xkp[b][1],
                "xv": xvp[b],
                "wq": wq_pack,
                "wvh": wvh,
                "wvl": wvl,
                "mask_a": masks[h][0],
                "mask_b": masks[h][1],
                "ones": ones_np,
                "scl": scl_np,
            }
        )

    nc = _get_nc()
    res = bass_utils.run_bass_kernel_spmd(
        nc, in_maps, core_ids=list(range(2 * B)), trace=trace
    )

    out = np.empty((B, S, D), dtype=np.float32)
    for c in range(2 * B):
        b, h = c // 2, c % 2
        o = np.asarray(res.results[c]["out"], dtype=np.float32)
        for s in range(NSLOT):
            out[b, 256 * s + 128 * h: 256 * s + 128 * h + P, :] = o[
                s * P:(s + 1) * P, :
            ]

    if trace:
        return out, res
    return out
